# revision 1
# baseline (speedup 1.0000x reference)
"""Multi-head causal self-attention on 8 Trainium2 NeuronCores.

Problem: X[4,2048,1024], per-head Wq/Wk/Wv[16,1024,64], Wo[1024,1024], bo[1024].
    out = OutProj(concat_heads(softmax_causal(Q K^T / 8) V))

Sharding: 8 cores = 4 batches x 2 head-groups (8 heads each). Each core
computes its batch's attention for its 8 heads plus the partial output
projection over its 512 concat features; host sums the two partials per
batch and adds the bias.

Per-core kernel (matmul operands in fp16 — 1 cycle/row on TensorE and
fp32 PSUM accumulation; softmax runs in the transposed
"feature-on-partition" space so its reduction lands on the free dim):
  qT/kT per head-pair  [128, T]  = Wpair^T  x  X^T
  v    per s-tile      [128, 8*65] = X^T^T  x  Wv_all (65th col set to 1)
  ST block [s=128, t=512] = kT_slice^T @ qT_slice   (row-packed head pairs)
  expST = exp(ST/8) (ScalarE), causal-masked via tri multiply
  avT [65, 512] += [V|1]^T @ expST   -> rows 0:64 = (A@V)^T, row 64 = sums
  normalize via 1/sums broadcast and write concatT
  partial = concatT^T @ WoST  (accumulated over 4 feature chunks)

Scheduling: everything is emitted in DMA-arrival order. X streams in
T-block order; pair 0's tt>0 projections are deferred into its attention
loop (need-forced per query tile) so the first scores fire as soon as
the first X block lands. Warmup matmuls fill the DMA window and keep the
PE HAM clock at 8/8. The output projection is interleaved into pair 3's
attention as PE filler; pair 3 normalizes in 128-column slices so each
out-proj group unblocks as early as possible.
"""

import os
import sys

for _p in ("/opt/trn_rl_repo", "/root/.axon_site/_ro/trn_rl_repo"):
    if os.path.isdir(_p) and _p not in sys.path:
        sys.path.append(_p)

import numpy as np

import concourse.mybir as mybir
import concourse.tile as tile
from concourse import bacc

B, T, D, H, K = 4, 2048, 1024, 16, 64
HG = 8          # heads per core
NPAIR = 4       # head pairs per core
P = 128
DC = D // P     # 8 contraction chunks for the projections
NS = T // P     # 16 key tiles
NT = T // 512   # 4 query tiles of 512
F32 = mybir.dt.float32
F16 = mybir.dt.float16


def build_module():
    nc = bacc.Bacc("TRN2")
    XT = nc.dram_tensor("xt", [D, T], F16, kind="ExternalInput").ap()
    WQ = nc.dram_tensor("wq", [NPAIR, D, P], F16, kind="ExternalInput").ap()
    WK = nc.dram_tensor("wk", [NPAIR, D, P], F16, kind="ExternalInput").ap()
    WV = nc.dram_tensor("wv", [D, HG * K], F16, kind="ExternalInput").ap()
    WO = nc.dram_tensor("wo", [HG * K, D], F16, kind="ExternalInput").ap()
    OUT = nc.dram_tensor("out", [T, D], F16, kind="ExternalOutput").ap()

    with tile.TileContext(nc) as tc:
        with tc.tile_pool(name="persist", bufs=1) as pp:
            xt_sb = pp.tile([P, DC, T], F16)            # X^T, 32 KB/partition
            v_sb = pp.tile([P, NS, HG * (K + 1)], F16)  # V + ones col per head
            concat_sb = pp.tile([P, NPAIR, T], F16)     # concat(heads)^T
            tri_sb = pp.tile([P, P], F16)   # causal triangle: 1 where x >= p
            warm_sb = pp.tile([P, 512], F16)

            # X^T in T-block order: the first 512 token-cols of every chunk
            # land first so V/Q/K tile 0 start as soon as possible. Each
            # DMA queue only sustains ~110-220 GB/s, so the critical
            # startup set (first X block + Wv + pair-0 Wq/Wk) is spread
            # over all three trigger queues; the X tail and Wo follow in
            # arrival-deadline order.
            xt_r = XT.rearrange("(c p) t -> c p t", p=P)
            wo_sb = pp.tile([P, NPAIR, D], F16)
            for c in range(DC):
                nc.sync.dma_start(out=xt_sb[:, c, 0:512], in_=xt_r[c][:, 0:512])
            nc.vector.memset(warm_sb, 0.0)
            nc.vector.memset(tri_sb, 1.0)
            nc.gpsimd.affine_select(
                out=tri_sb,
                in_=tri_sb,
                compare_op=mybir.AluOpType.is_ge,
                fill=0.0,
                base=0,
                channel_multiplier=-1,
                pattern=[[1, P]],
            )
            # ones column (index 64 of each head's 65-wide slot)
            v_slots = v_sb.rearrange("p s (h x) -> p s h x", x=K + 1)
            nc.vector.memset(v_slots[:, :, :, K : K + 1], 1.0)

            # ---- V + Q/K projections + attention, software-pipelined ----
            # Projection matmuls (V tail, pair-0 tt>0, the NEXT pair's Q/K)
            # and pair-3's output projection are interleaved into the
            # attention loop so the PE fills the stalls where it would
            # otherwise wait on ScalarE's exp.
            with (
                tc.tile_pool(name="wvp", bufs=1) as wvp,
                tc.tile_pool(name="attn", bufs=1) as ap_,
            ):
                psa = None  # assigned after the startup PSUM pool closes
                wv_sb = wvp.tile([P, DC, HG * K], F16)
                wv_r = WV.rearrange("(c p) n -> c p n", p=P)
                # (wv DMA is emitted after pair-0's wq/wk: the first scores
                # need only Q/K + the first X block, so those bytes go first)

                def v_group_ops(s, pool=None, bufs=2):
                    ps = (pool or psa).tile(
                        [P, HG * K], F32, tag="mm", bufs=bufs, name=f"vps{s}"
                    )
                    ops = [
                        (
                            lambda c=c, ps=ps, s=s: nc.tensor.matmul(
                                ps,
                                xt_sb[:, c, s * P : (s + 1) * P],
                                wv_sb[:, c, :],
                                start=(c == 0),
                                stop=(c == DC - 1),
                            )
                        )
                        for c in range(DC)
                    ]
                    ops.append(
                        lambda ps=ps, s=s: nc.vector.tensor_copy(
                            v_slots[:, s, :, 0:K],
                            ps.rearrange("p (h k) -> p h k", k=K),
                        )
                    )
                    return ops

                def proj_weights(pr, split=False):
                    """DMA the pair's Wq/Wk; alloc the q/k destinations."""
                    wq_sb = ap_.tile(
                        [P, DC, P], F16, tag="wq", bufs=2, name=f"wq{pr}"
                    )
                    wk_sb = ap_.tile(
                        [P, DC, P], F16, tag="wk", bufs=2, name=f"wk{pr}"
                    )
                    nc.scalar.dma_start(
                        out=wq_sb, in_=WQ[pr].rearrange("(c p) m -> p c m", p=P)
                    )
                    (nc.gpsimd if split else nc.scalar).dma_start(
                        out=wk_sb, in_=WK[pr].rearrange("(c p) m -> p c m", p=P)
                    )
                    q_sb = ap_.tile([P, T], F16, tag="q", bufs=2, name=f"q{pr}")
                    k_sb = ap_.tile([P, T], F16, tag="k", bufs=2, name=f"k{pr}")
                    return wq_sb, wk_sb, q_sb, k_sb

                def proj_ops(pr, tiles, tts, pool=None, bufs=2):
                    """Projection matmuls in tt-major (DMA-arrival) order."""
                    wq_sb, wk_sb, q_sb, k_sb = tiles
                    ops = []
                    for tt in tts:
                        for w_sb, qk_sb, nm in (
                            (wq_sb, q_sb, "q"),
                            (wk_sb, k_sb, "k"),
                        ):
                            ps = (pool or psa).tile(
                                [P, 512], F32, tag="mm", bufs=bufs,
                                name=f"{nm}ps{pr}_{tt}",
                            )
                            for c in range(DC):
                                ops.append(
                                    lambda ps=ps, w_sb=w_sb, c=c, tt=tt:
                                    nc.tensor.matmul(
                                        ps,
                                        w_sb[:, c, :],
                                        xt_sb[
                                            :, c, tt * 512 : (tt + 1) * 512
                                        ],
                                        start=(c == 0),
                                        stop=(c == DC - 1),
                                    )
                                )
                            ops.append(
                                lambda ps=ps, qk_sb=qk_sb, tt=tt:
                                nc.vector.tensor_copy(
                                    qk_sb[:, tt * 512 : (tt + 1) * 512], ps
                                )
                            )
                    return ops

                flush_mode = [False]

                def op_group_ops(t16, oc):
                    """Output-projection group for one [128 t, 512 oc] tile."""
                    holder = {}

                    def mm(s4):
                        def f():
                            if "ps" not in holder:
                                holder["ps"] = psa.tile(
                                    [P, 512], F32, tag="mm", bufs=2,
                                    name=f"ops{t16}_{oc}",
                                )
                            nc.tensor.matmul(
                                holder["ps"],
                                concat_sb[:, s4, t16 * P : (t16 + 1) * P],
                                wo_sb[:, s4, oc * 512 : (oc + 1) * 512],
                                start=(s4 == 0),
                                stop=(s4 == NPAIR - 1),
                            )
                        return f

                    def fin():
                        st_o = ap_.tile(
                            [P, 512], F16, tag="outst", bufs=6,
                            name=f"ost{t16}_{oc}",
                        )
                        if flush_mode[0] and (t16 + oc) % 2 == 0:
                            # ScalarE is exp-free during the final flush;
                            # alternate with DVE so the copies pipeline
                            nc.scalar.copy(st_o, holder["ps"])
                        else:
                            nc.vector.tensor_copy(st_o, holder["ps"])
                        if flush_mode[0]:
                            eng = nc.sync if oc == 0 else nc.scalar
                        else:
                            eng = nc.sync if oc == 0 else nc.gpsimd
                        eng.dma_start(
                            out=OUT[
                                t16 * P : (t16 + 1) * P,
                                oc * 512 : (oc + 1) * 512,
                            ],
                            in_=st_o,
                        )

                    return [mm(s4) for s4 in range(NPAIR)] + [fin]

                def drain_avs(pr, tt, avs):
                    """Free the 2-slot avs PSUM rotation fast (the next tt's
                    first AV matmul waits on it): one bulk copy per head,
                    then the whole normalize chain runs off the copy."""
                    cps, scp = [], []
                    for h2 in range(2):
                        cp = ap_.tile(
                            [K + 1, 512], F32, tag="avcp", bufs=4,
                            name=f"avcp{pr}_{tt}_{h2}",
                        )
                        if h2 == 1:
                            # ScalarE is exp-idle at pair/tail boundaries
                            nc.scalar.copy(cp, avs[h2])
                        else:
                            nc.vector.tensor_copy(cp, avs[h2])
                        cps.append(cp)
                    for h2 in range(2):
                        sp = ap_.tile(
                            [1, 512], F32, tag="scp", bufs=4,
                            name=f"scp{pr}_{tt}_{h2}",
                        )
                        nc.vector.tensor_copy(sp, cps[h2][K : K + 1, :])
                        scp.append(sp)
                    return cps, scp

                def direct_normalize(pr, tt, avs):
                    """Whole-tile normalize straight from the avs PSUM."""
                    for h in range(2):
                        cols = slice(tt * 512, (tt + 1) * 512)
                        sums = ap_.tile([1, 512], F32, tag="sums", bufs=6)
                        nc.vector.tensor_copy(sums, avs[h][K : K + 1, :])
                        recip = ap_.tile([1, 512], F32, tag="recip", bufs=6)
                        nc.vector.reciprocal_approx_fast(recip, sums)
                        bc_sb = ap_.tile([K, 512], F32, tag="bc_sb", bufs=6)
                        nc.gpsimd.partition_broadcast(bc_sb, recip)
                        if h == 0:
                            dst = concat_sb[0:K, pr, cols]
                        else:
                            dst = ap_.tile([K, 512], F16, tag="tmpb", bufs=6)
                        nc.vector.tensor_mul(dst, avs[h][0:K, :], bc_sb)
                        if h == 1:
                            nc.sync.dma_start(
                                out=concat_sb[K:P, pr, cols], in_=dst
                            )

                def normalize(pr, tt, cps, scp, cols_lo, width,
                              bounce_eng=None):
                    """1/sums for both heads over [cols_lo, cols_lo+width)."""
                    for h in range(2):
                        cols = slice(tt * 512 + cols_lo, tt * 512 + cols_lo + width)
                        psl = slice(cols_lo, cols_lo + width)
                        recip = ap_.tile([1, width], F32, tag="recip", bufs=6)
                        nc.vector.reciprocal_approx_fast(recip, scp[h][0:1, psl])
                        bc_sb = ap_.tile([K, width], F32, tag="bc_sb", bufs=6)
                        nc.gpsimd.partition_broadcast(bc_sb, recip)
                        if h == 0:
                            dst = concat_sb[0:K, pr, cols]
                        else:
                            dst = ap_.tile([K, width], F16, tag="tmpb", bufs=6)
                        nc.vector.tensor_mul(dst, cps[h][0:K, psl], bc_sb)
                        if h == 1:
                            # partition-shifted write via DMA bounce
                            (bounce_eng or nc.sync).dma_start(
                                out=concat_sb[K:P, pr, cols], in_=dst
                            )

                # upfront: warmup matmuls fill the X-DMA window and get the
                # PE HAM clock to 8/8 before real work; then V s0-3 and
                # pair 0's tt=0 Q/K — exactly what the first X block feeds.
                tiles0 = proj_weights(0, split=True)
                for c in range(DC):
                    (nc.scalar if c < 4 else nc.gpsimd).dma_start(
                        out=wv_sb[:, c, :], in_=wv_r[c]
                    )
                # X tail, emitted after the critical startup set so the
                # queue order matches arrival deadlines (tt1 needs
                # 512:1280 by ~25us, tt2 1024:1536 by ~35us, ...)
                for c in range(DC):
                    nc.sync.dma_start(
                        out=xt_sb[:, c, 512:1280], in_=xt_r[c][:, 512:1280]
                    )
                for c in range(DC):
                    (nc.scalar if c % 2 else nc.gpsimd).dma_start(
                        out=xt_sb[:, c, 1280:2048], in_=xt_r[c][:, 1280:2048]
                    )
                nc.gpsimd.dma_start(
                    out=wo_sb, in_=WO.rearrange("(s p) o -> p s o", p=P)
                )
                with tc.tile_pool(name="ps0", bufs=1, space="PSUM") as ps0:
                    warm_ps = ps0.tile([P, 512], F32, tag="warm", bufs=1)

                    def warm(n):
                        # dependency-free matmuls: keep the PE HAM activity
                        # window busy while real matmuls are DMA-paced
                        for _ in range(n):
                            nc.tensor.matmul(
                                warm_ps, warm_sb[:, 0:P], warm_sb,
                                start=True, stop=True,
                            )

                    warm(4)
                    # Q/K tile 0 first: the first scores need only these +
                    # the first X block; the V groups then fill the PE
                    # while the exp stream spins up
                    for op in proj_ops(0, tiles0, [0], pool=ps0, bufs=6):
                        op()
                    for s in range(4):
                        for op in v_group_ops(s, pool=ps0, bufs=6):
                            op()
                psa_cm = tc.tile_pool(name="psa", bufs=1, space="PSUM")
                psa = psa_cm.__enter__()
                _, _, q_sb, k_sb = tiles0
                vqueue = [op for s in range(4, NS) for op in v_group_ops(s)]
                qk0queue = proj_ops(0, tiles0, [1, 2, 3])
                pending = []
                opqueue = []
                v_done = [0]
                qk0_done = [0]

                def score_exp(qs, ks, tt, si, tag, bufs):
                    """Score pair + exp + causal mask for one key tile."""
                    m = si - 4 * tt
                    off = max(m, 0) * P
                    nv = 512 - off
                    st = psa.tile([P, 2, 512], F32, tag="stw", bufs=2)
                    ex = ap_.tile(
                        [P, 2, 512], F16, tag=tag, bufs=bufs,
                        name=f"{tag}{tt}_{si}",
                    )
                    for h in range(2):
                        lo, hi = h * K, (h + 1) * K
                        nc.tensor.matmul(
                            st[:, h, 0:nv],
                            ks[lo:hi, si * P : (si + 1) * P],
                            qs[lo:hi, tt * 512 + off : (tt + 1) * 512],
                            start=True,
                            stop=True,
                            tile_position=(lo, 0),
                        )
                    nc.scalar.activation(
                        ex[:, :, 0:nv], st[:, :, 0:nv],
                        mybir.ActivationFunctionType.Exp,
                        scale=0.125,
                    )
                    if m >= 0:  # mask both heads' leading triangles
                        nc.vector.tensor_mul(
                            ex[:, :, 0:P],
                            ex[:, :, 0:P],
                            tri_sb.unsqueeze(1).broadcast_to([P, 2, P]),
                        )
                    return ex

                def pop_one(allow_op):
                    if vqueue:
                        vqueue.pop(0)()
                        v_done[0] += 1
                    elif qk0queue:
                        qk0queue.pop(0)()
                        qk0_done[0] += 1
                    elif pending:
                        pending.pop(0)()
                    elif allow_op and opqueue:
                        opqueue.pop(0)()

                for pr in range(NPAIR):
                    if pr < NPAIR - 1:
                        ntiles = proj_weights(pr + 1)
                        pending.extend(proj_ops(pr + 1, ntiles, range(NT)))
                    si_left = sum(4 * tt + 4 for tt in range(NT))

                    for tt in range(NT):
                        if pr == 0:
                            # V for this tt's key tiles and this tt's q/k
                            # chunks must be in flight before attention
                            need = (4 * tt) * 9
                            while v_done[0] < need and vqueue:
                                vqueue.pop(0)()
                                v_done[0] += 1
                            need_qk = 18 * tt
                            while qk0_done[0] < need_qk and qk0queue:
                                qk0queue.pop(0)()
                                qk0_done[0] += 1
                        avs = [
                            psa.tile(
                                [K + 1, 512], F32, tag="av", bufs=2,
                                name=f"av{pr}_{tt}_{h2}",
                            )
                            for h2 in range(2)
                        ]
                        n_s = 4 * tt + 4
                        for si in range(n_s):
                            # adaptive fill rate: spread the queued filler
                            # matmuls evenly over the pair's remaining steps
                            nq = (
                                len(vqueue) + len(qk0queue) + len(pending)
                                + len(opqueue)
                            )
                            pops = min(
                                6,
                                max(
                                    -(-nq // max(1, si_left)),
                                    3 if opqueue else 0,
                                ),
                            )
                            for _ in range(pops):
                                # op groups wait 2 steps so their concat
                                # DMA bounce never head-of-line blocks
                                pop_one(allow_op=(si >= 2))
                            si_left -= 1
                            # diagonal blocks: only cols >= 128*m can be valid
                            m = si - 4 * tt
                            off = max(m, 0) * P
                            nv = 512 - off
                            if pr >= 1 and tt == 0:
                                # scores+exp for this tile were pre-computed
                                # during the previous pair's tail
                                ex = pre_ex[si]
                            else:
                                ex = score_exp(q_sb, k_sb, tt, si, "exp", 10)
                            for h in range(2):
                                slot = (2 * pr + h) * (K + 1)
                                nc.tensor.matmul(
                                    avs[h][:, off:512],
                                    v_sb[:, si, slot : slot + K + 1],
                                    ex[:, h, 0:nv],
                                    start=(si == 0),
                                    stop=(si == n_s - 1),
                                )
                        # Free the AV accumulators and normalize. Pair 3
                        # drains PSUM with bulk copies and normalizes in
                        # 128-col slices (bounces spread over two DMA
                        # queues) so each out-proj group unblocks early;
                        # pairs 0-2 normalize straight from PSUM.
                        if pr == NPAIR - 1:
                            cps, scp = drain_avs(pr, tt, avs)
                            for i16 in range(4):
                                normalize(
                                    pr, tt, cps, scp, i16 * P, P,
                                    bounce_eng=(
                                        nc.sync if i16 % 2 == 0 else nc.gpsimd
                                    ),
                                )
                                for oc in range(2):
                                    opqueue.extend(
                                        op_group_ops(4 * tt + i16, oc)
                                    )
                        else:
                            direct_normalize(pr, tt, avs)

                    # next pair's projections must be complete before its
                    # attention starts; flush whatever wasn't interleaved
                    while vqueue or qk0queue or pending:
                        pop_one(allow_op=False)
                    if pr < NPAIR - 1:
                        # migrate the next pair's tt0 scores+exp into this
                        # pair's tail: each migration frees ~4.6us of exp
                        # from the next ScalarE-bound phase, which in turn
                        # creates the slack that hosts the one after it
                        nq_sb, nk_sb = ntiles[2], ntiles[3]
                        pre_ex = [
                            score_exp(nq_sb, nk_sb, 0, si, "exq", 4)
                            for si in range(4)
                        ]
                    if pr < NPAIR - 1:
                        _, _, q_sb, k_sb = ntiles
                # bridge the final normalize chain: keep the PE busy so HAM
                # stays warm and the flush matmuls run at full clock
                warmf = psa.tile([P, 512], F32, tag="mm", bufs=2, name="warmf")
                for _ in range(10):
                    nc.tensor.matmul(
                        warmf, warm_sb[:, 0:P], warm_sb, start=True, stop=True
                    )
                flush_mode[0] = True
                while opqueue:
                    opqueue.pop(0)()
                psa_cm.__exit__(None, None, None)
    _fuse_score_ldweights(nc)
    nc.compile()
    return nc


def _fuse_score_ldweights(nc):
    """Merge each score pair's two 64-row LDWEIGHTS into one 128-row load.

    The post-Tile IR carries [Ldw(h0 64p), MM(0,0), Ldw(h1 64p), MM(64,0)]
    per key tile. With two LDWs the PE stalls ~100ns on each side of the
    pair (single background weight buffer). One 128-row LDW loads both
    heads' K slice at once; the row-tiled matmuls then address their own
    row groups of the already-loaded array.
    """
    fn = list(nc.m.functions)[0]
    fused = 0
    for blk in fn.blocks:
        insts = blk.instructions
        # pattern-match on the PE-engine subsequence: other engines'
        # instructions interleave freely in the block list
        pe = [
            (i, x)
            for i, x in enumerate(insts)
            if type(x).__name__ in ("InstLdweights", "InstMatmult")
        ]
        drop = []
        for k in range(len(pe) - 3):
            (_, a), (_, b), (ic, c), (_, d) = pe[k], pe[k + 1], pe[k + 2], pe[k + 3]
            if not (
                type(a).__name__ == "InstLdweights"
                and type(b).__name__ == "InstMatmult"
                and type(c).__name__ == "InstLdweights"
                and type(d).__name__ == "InstMatmult"
            ):
                continue
            if not (
                tuple(b.tile_size or ()) == (64, 128)
                and tuple(b.tile_position or ()) == (0, 0)
                and tuple(d.tile_size or ()) == (64, 128)
                and tuple(d.tile_position or ()) == (64, 0)
            ):
                continue
            apA, apC = a.ins[0], c.ins[0]
            pa, pc = list(apA.ap), list(apC.ap)
            if not (
                len(pa) == 2
                and pa[0][1] == 64
                and pc[0][1] == 64
                and pa[0][0] == pc[0][0]
                and pa[1] == pc[1]
                and apC.offset == apA.offset + 64 * pa[0][0]
                and c.sync_info is None
            ):
                continue
            apA.ap = [[pa[0][0], 128], pa[1]]
            if tuple(a.tile_size or ()) == (64, 128):
                a.tile_size = (128, 128)
            a.merge_dependencies_from(c)
            drop.append(ic)
            fused += 1
        for j in sorted(drop, reverse=True):
            del insts[j]
    assert fused > 0, "score LDW fusion matched nothing"


def shard_inputs(X, Wq, Wk, Wv, Wo):
    """Host-side shard prep: core c handles batch c//2, head group c%2."""
    in_maps = []
    for c in range(8):
        b, g = c // 2, c % 2
        heads = range(g * HG, (g + 1) * HG)
        wq = np.stack(
            [
                np.concatenate([Wq[g * HG + 2 * p], Wq[g * HG + 2 * p + 1]], axis=1)
                for p in range(NPAIR)
            ]
        )
        wk = np.stack(
            [
                np.concatenate([Wk[g * HG + 2 * p], Wk[g * HG + 2 * p + 1]], axis=1)
                for p in range(NPAIR)
            ]
        )
        wv = np.concatenate([Wv[h] for h in heads], axis=1)
        wo = Wo[:, g * 512 : (g + 1) * 512].T
        in_maps.append(
            {
                "xt": np.ascontiguousarray(X[b].T).astype(np.float16),
                "wq": np.ascontiguousarray(wq).astype(np.float16),
                "wk": np.ascontiguousarray(wk).astype(np.float16),
                "wv": np.ascontiguousarray(wv).astype(np.float16),
                "wo": np.ascontiguousarray(wo).astype(np.float16),
            }
        )
    return in_maps


_MODULE = None


def _get_module():
    global _MODULE
    if _MODULE is None:
        _MODULE = build_module()
    return _MODULE


def kernel(X, Wq, Wk, Wv, Wo, bo, _want_results=None):
    from concourse.bass_utils import run_bass_kernel_spmd

    nc = _get_module()
    in_maps = shard_inputs(
        np.asarray(X), np.asarray(Wq), np.asarray(Wk), np.asarray(Wv), np.asarray(Wo)
    )
    res = run_bass_kernel_spmd(nc, in_maps, core_ids=list(range(8)))
    if _want_results is not None:
        _want_results.append(res)
    out = np.empty((B, T, H * K), dtype=np.float32)
    bo = np.asarray(bo, dtype=np.float32)
    for b in range(B):
        out[b] = (
            res.results[2 * b]["out"].astype(np.float32)
            + res.results[2 * b + 1]["out"].astype(np.float32)
            + bo
        )
    return out



# revision 11
# speedup vs baseline: 1.0264x; 1.0264x over previous
"""Multi-head causal self-attention on 8 Trainium2 NeuronCores.

Problem: X[4,2048,1024], per-head Wq/Wk/Wv[16,1024,64], Wo[1024,1024], bo[1024].
    out = OutProj(concat_heads(softmax_causal(Q K^T / 8) V))

Sharding: 8 cores = 4 batches x 2 head-groups (8 heads each). Each core
computes its batch's attention for its 8 heads plus the partial output
projection over its 512 concat features; host sums the two partials per
batch and adds the bias.

Per-core kernel (matmul operands in fp16 — 1 col/cycle on TensorE with
fp32 PSUM accumulation; softmax runs in the transposed
"feature-on-partition" space so its reduction lands on the free dim):
  qT/kT per head-pair  [128, T]  = Wpair^T  x  X^T
  v    per s-tile      [128, 8*65] = X^T^T  x  Wv_all (65th col set to 1)
  ST block [s=128, t=512] = kT_slice^T @ qT_slice   (row-packed head pairs:
     the two 64-row tiles share one fused LDWEIGHTS and stream their
     moving operands CONCURRENTLY — disjoint SBUF partitions + disjoint
     PSUM banks — so a score pair costs ~nv cycles, not 2*nv)
  expST = exp(ST/8) (ScalarE), causal-masked via tri multiply
  avT [65, 512] += [V|1]^T @ expST   -> rows 0:64 = (A@V)^T, row 64 = sums
  normalize via 1/sums broadcast and write concatT
  partial = concatT^T @ WoST  (accumulated over 4 feature chunks)

Schedule (v2): tt-MAJOR — for each 512-wide query tile tt, all four head
pairs run their attention segment back-to-back.  All pairs' Q/K live in
SBUF simultaneously, so the output projection for query block tt unlocks
as soon as phase tt completes (25/50/75/100% marks) instead of piling
into the last quarter.  Fill work (later-phase projections, V tail,
out-proj groups) is interleaved into the attention stream under a
simple clock model of PE vs ScalarE so the in-order PE queue never
head-of-line blocks on an exp that hasn't fired: per si step the AV
matmuls trail the score pair by one step, and filler is popped until
the PE clock catches the predicted exp completion.
"""

import os
import sys

for _p in ("/opt/trn_rl_repo", "/root/.axon_site/_ro/trn_rl_repo"):
    if os.path.isdir(_p) and _p not in sys.path:
        sys.path.append(_p)

import numpy as np

import concourse.mybir as mybir
import concourse.tile as tile
from concourse import bacc

B, T, D, H, K = 4, 2048, 1024, 16, 64
HG = 8          # heads per core
NPAIR = 4       # head pairs per core
P = 128
DC = D // P     # 8 contraction chunks for the projections
NS = T // P     # 16 key tiles
NT = T // 512   # 4 query tiles of 512
F32 = mybir.dt.float32
F16 = mybir.dt.float16

# clock-model constants (ns), calibrated from the v1 trace
MM_NS = 216.0 / 512.0      # per streamed column, 512-col mm ~216ns cadence
PAIR_FIX = 100.0           # extra fixed cost of a score-pair issue
EXP_COL = 0.87             # ScalarE ns per column
EXP_FIX = 260.0            # ScalarE per-activation overhead
EXP_LAG = 220.0            # sem propagation mm-done -> exp start


def build_module():
    nc = bacc.Bacc("TRN2")
    XT = nc.dram_tensor("xt", [D, T], F16, kind="ExternalInput").ap()
    WQ = nc.dram_tensor("wq", [NPAIR, D, P], F16, kind="ExternalInput").ap()
    WK = nc.dram_tensor("wk", [NPAIR, D, P], F16, kind="ExternalInput").ap()
    WV = nc.dram_tensor("wv", [D, HG * K], F16, kind="ExternalInput").ap()
    WO = nc.dram_tensor("wo", [HG * K, D], F16, kind="ExternalInput").ap()
    OUT = nc.dram_tensor("out", [T, D], F16, kind="ExternalOutput").ap()

    with tile.TileContext(nc) as tc:
        with tc.tile_pool(name="persist", bufs=1) as pp:
            xt_sb = pp.tile([P, DC, T], F16)            # X^T, 32 KB/partition
            v_sb = pp.tile([P, NS, HG * (K + 1)], F16)  # V + ones col per head
            concat_sb = pp.tile([P, NPAIR, T], F16)     # concat(heads)^T
            tri_sb = pp.tile([P, P], F16)   # causal triangle: 1 where x >= p
            warm_sb = pp.tile([P, 512], F16)
            wo_sb = pp.tile([P, NPAIR, D], F16)
            wv_sb = pp.tile([P, DC, HG * K], F16)
            wq_sb = [pp.tile([P, DC, P], F16, name=f"wq{p}") for p in range(NPAIR)]
            wk_sb = [pp.tile([P, DC, P], F16, name=f"wk{p}") for p in range(NPAIR)]
            q_sb = [pp.tile([P, T], F16, name=f"q{p}") for p in range(NPAIR)]
            k_sb = [pp.tile([P, T], F16, name=f"k{p}") for p in range(NPAIR)]

            xt_r = XT.rearrange("(c p) t -> c p t", p=P)
            xt_p = XT.rearrange("(c p) t -> p c t", p=P)
            wv_p = WV.rearrange("(c p) n -> p c n", p=P)
            wo_p = WO.rearrange("(s p) o -> p s o", p=P)

            # ---- DMA priority emission -------------------------------
            # sync queue: X block 0 chunk-by-chunk (separate completion
            # sems so projection chunk c can fire as soon as chunk c
            # lands), then blocks 1-2 as merged half transfers.
            for c in range(DC):
                nc.sync.dma_start(out=xt_sb[:, c, 0:512], in_=xt_r[c][:, 0:512])
            # scalar queue: all pairs' Wq (tt-major needs every pair
            # early), then half of Wv.  Scalar issues these before its
            # first exp, then stays exp-pure.
            for pr in range(NPAIR):
                nc.scalar.dma_start(
                    out=wq_sb[pr], in_=WQ[pr].rearrange("(c p) m -> p c m", p=P)
                )
            # gpsimd queue: all pairs' Wk, then the other half of Wv.
            for pr in range(NPAIR):
                nc.gpsimd.dma_start(
                    out=wk_sb[pr], in_=WK[pr].rearrange("(c p) m -> p c m", p=P)
                )
            nc.scalar.dma_start(out=wv_sb[:, 0:4, :], in_=wv_p[:, 0:4, :])
            nc.gpsimd.dma_start(out=wv_sb[:, 4:8, :], in_=wv_p[:, 4:8, :])
            # X tail in arrival-deadline order
            nc.sync.dma_start(
                out=xt_sb[:, 0:4, 512:1024], in_=xt_p[:, 0:4, 512:1024]
            )
            nc.sync.dma_start(
                out=xt_sb[:, 4:8, 512:1024], in_=xt_p[:, 4:8, 512:1024]
            )
            nc.scalar.dma_start(out=wo_sb[:, 0:2, :], in_=wo_p[:, 0:2, :])
            nc.gpsimd.dma_start(out=wo_sb[:, 2:4, :], in_=wo_p[:, 2:4, :])
            nc.sync.dma_start(
                out=xt_sb[:, 0:4, 1024:1536], in_=xt_p[:, 0:4, 1024:1536]
            )
            nc.sync.dma_start(
                out=xt_sb[:, 4:8, 1024:1536], in_=xt_p[:, 4:8, 1024:1536]
            )
            nc.gpsimd.dma_start(
                out=xt_sb[:, 0:4, 1536:2048], in_=xt_p[:, 0:4, 1536:2048]
            )
            nc.gpsimd.dma_start(
                out=xt_sb[:, 4:8, 1536:2048], in_=xt_p[:, 4:8, 1536:2048]
            )

            nc.vector.memset(warm_sb, 0.0)
            nc.vector.memset(tri_sb, 1.0)
            nc.gpsimd.affine_select(
                out=tri_sb,
                in_=tri_sb,
                compare_op=mybir.AluOpType.is_ge,
                fill=0.0,
                base=0,
                channel_multiplier=-1,
                pattern=[[1, P]],
            )
            # ones column (index 64 of each head's 65-wide slot)
            v_slots = v_sb.rearrange("p s (h x) -> p s h x", x=K + 1)
            nc.vector.memset(v_slots[:, :, :, K : K + 1], 1.0)

            with (
                tc.tile_pool(name="attn", bufs=1) as ap_,
                tc.tile_pool(name="psa", bufs=1, space="PSUM") as psa,
            ):
                # ---------------- op builders -------------------------
                def v_group_ops(s):
                    """V projection for one key tile: 8 mms + 1 cast."""
                    holder = {}

                    def mm(c):
                        def f():
                            if "ps" not in holder:
                                holder["ps"] = psa.tile(
                                    [P, HG * K], F32, tag="mm", bufs=2,
                                    name=f"vps{s}",
                                )
                            nc.tensor.matmul(
                                holder["ps"],
                                xt_sb[:, c, s * P : (s + 1) * P],
                                wv_sb[:, c, :],
                                start=(c == 0),
                                stop=(c == DC - 1),
                            )
                        return f

                    def fin():
                        nc.vector.tensor_copy(
                            v_slots[:, s, :, 0:K],
                            holder["ps"].rearrange("p (h k) -> p h k", k=K),
                        )

                    return [(mm(c), MM_NS * 512) for c in range(DC)] + [(fin, 0.0)]

                def proj_unit_ops(pr, tt, which):
                    """Q or K projection for (pair, query tile): 8 mms+cast."""
                    w_sb = wq_sb[pr] if which == "q" else wk_sb[pr]
                    dst = q_sb[pr] if which == "q" else k_sb[pr]
                    holder = {}

                    def mm(c):
                        def f():
                            if "ps" not in holder:
                                holder["ps"] = psa.tile(
                                    [P, 512], F32, tag="mm", bufs=2,
                                    name=f"{which}ps{pr}_{tt}",
                                )
                            nc.tensor.matmul(
                                holder["ps"],
                                w_sb[:, c, :],
                                xt_sb[:, c, tt * 512 : (tt + 1) * 512],
                                start=(c == 0),
                                stop=(c == DC - 1),
                            )
                        return f

                    def fin():
                        nc.vector.tensor_copy(
                            dst[:, tt * 512 : (tt + 1) * 512], holder["ps"]
                        )

                    return [(mm(c), MM_NS * 512) for c in range(DC)] + [(fin, 0.0)]

                out_q = [nc.gpsimd, nc.sync]
                out_qi = [0]
                flush_mode = [False]

                def op_group_ops(t16, oc):
                    """Output-projection group for one [128 t, 512 oc] tile."""
                    holder = {}

                    def mm(s4):
                        def f():
                            if "ps" not in holder:
                                holder["ps"] = psa.tile(
                                    [P, 512], F32, tag="mm", bufs=2,
                                    name=f"ops{t16}_{oc}",
                                )
                            nc.tensor.matmul(
                                holder["ps"],
                                concat_sb[:, s4, t16 * P : (t16 + 1) * P],
                                wo_sb[:, s4, oc * 512 : (oc + 1) * 512],
                                start=(s4 == 0),
                                stop=(s4 == NPAIR - 1),
                            )
                        return f

                    def fin():
                        st_o = ap_.tile(
                            [P, 512], F16, tag="outst", bufs=6,
                            name=f"ost{t16}_{oc}",
                        )
                        if flush_mode[0] and (t16 + oc) % 2 == 0:
                            nc.scalar.copy(st_o, holder["ps"])
                        else:
                            nc.vector.tensor_copy(st_o, holder["ps"])
                        eng = out_q[out_qi[0] % len(out_q)]
                        out_qi[0] += 1
                        eng.dma_start(
                            out=OUT[
                                t16 * P : (t16 + 1) * P,
                                oc * 512 : (oc + 1) * 512,
                            ],
                            in_=st_o,
                        )

                    return [(mm(s4), MM_NS * 512) for s4 in range(NPAIR)] + [
                        (fin, 0.0)
                    ]

                # ---------------- fill queue --------------------------
                # (deadline, cost_ns, fn); deadline = phase index by which
                # the op must have run (checked at phase starts).
                fillq = []

                def fill_extend(deadline, ops):
                    for fn, cost in ops:
                        fillq.append([deadline, cost, fn])

                clock = {"pe": 9000.0, "sc": 9000.0}
                sc_done = {}
                exp_hist = [0.0, 0.0]  # completion of last two exps (global)

                def pop_fill(n=1):
                    for _ in range(n):
                        if not fillq:
                            return False
                        _, cost, fn = fillq.pop(0)
                        fn()
                        clock["pe"] += cost
                    return True

                def pace_to(target):
                    while clock["pe"] < target and fillq:
                        pop_fill()

                def flush_due(phase):
                    while fillq and fillq[0][0] <= phase:
                        pop_fill()

                # ---------------- attention pieces --------------------
                def score_exp(pr, tt, si):
                    m = si - 4 * tt
                    off = max(m, 0) * P
                    nv = 512 - off
                    st = psa.tile([P, 2, 512], F32, tag="stw", bufs=2)
                    ex = ap_.tile(
                        [P, 2, 512], F16, tag="exp", bufs=8,
                        name=f"exp{pr}_{tt}_{si}",
                    )
                    for h in range(2):
                        lo, hi = h * K, (h + 1) * K
                        nc.tensor.matmul(
                            st[:, h, 0:nv],
                            k_sb[pr][lo:hi, si * P : (si + 1) * P],
                            q_sb[pr][lo:hi, tt * 512 + off : (tt + 1) * 512],
                            start=True,
                            stop=True,
                            tile_position=(lo, 0),
                        )
                    clock["pe"] += MM_NS * nv + PAIR_FIX
                    nc.scalar.activation(
                        ex[:, :, 0:nv], st[:, :, 0:nv],
                        mybir.ActivationFunctionType.Exp,
                        scale=0.125,
                    )
                    start = max(clock["sc"], clock["pe"] + EXP_LAG)
                    clock["sc"] = start + 2 * nv * EXP_COL + EXP_FIX
                    sc_done[(pr, tt, si)] = clock["sc"]
                    exp_hist.append(clock["sc"])
                    if m >= 0:  # mask both heads' leading triangles
                        nc.vector.tensor_mul(
                            ex[:, :, 0:P],
                            ex[:, :, 0:P],
                            tri_sb.unsqueeze(1).broadcast_to([P, 2, P]),
                        )
                    return ex, nv, off

                def av_pair(pr, tt, si, ex, nv, off, n_s):
                    for h in range(2):
                        slot = (2 * pr + h) * (K + 1)
                        nc.tensor.matmul(
                            avs[h][:, off:512],
                            v_sb[:, si, slot : slot + K + 1],
                            ex[:, h, 0:nv],
                            start=(si == 0),
                            stop=(si == n_s - 1),
                        )
                    clock["pe"] += 2 * MM_NS * nv

                def direct_normalize(pr, tt, avs):
                    for h in range(2):
                        cols = slice(tt * 512, (tt + 1) * 512)
                        sums = ap_.tile([1, 512], F32, tag="sums", bufs=6)
                        nc.vector.tensor_copy(sums, avs[h][K : K + 1, :])
                        recip = ap_.tile([1, 512], F32, tag="recip", bufs=6)
                        nc.vector.reciprocal_approx_fast(recip, sums)
                        bc_sb = ap_.tile([K, 512], F32, tag="bc_sb", bufs=6)
                        nc.gpsimd.partition_broadcast(bc_sb, recip)
                        if h == 0:
                            dst = concat_sb[0:K, pr, cols]
                        else:
                            dst = ap_.tile([K, 512], F16, tag="tmpb", bufs=6)
                        nc.vector.tensor_mul(dst, avs[h][0:K, :], bc_sb)
                        if h == 1:
                            nc.sync.dma_start(
                                out=concat_sb[K:P, pr, cols], in_=dst
                            )

                def slice_normalize(pr, tt, avs, i16):
                    """128-col slice normalize for the phase's last pair,
                    so each out-proj group unlocks as early as possible."""
                    cols_lo = i16 * P
                    for h in range(2):
                        cols = slice(tt * 512 + cols_lo, tt * 512 + cols_lo + P)
                        psl = slice(cols_lo, cols_lo + P)
                        sums = ap_.tile([1, P], F32, tag="sums", bufs=6)
                        nc.vector.tensor_copy(sums, avs[h][K : K + 1, psl])
                        recip = ap_.tile([1, P], F32, tag="recip", bufs=6)
                        nc.vector.reciprocal_approx_fast(recip, sums)
                        bc_sb = ap_.tile([K, P], F32, tag="bc_sb", bufs=6)
                        nc.gpsimd.partition_broadcast(bc_sb, recip)
                        if h == 0:
                            dst = concat_sb[0:K, pr, cols]
                        else:
                            dst = ap_.tile([K, P], F16, tag="tmpb", bufs=6)
                        nc.vector.tensor_mul(dst, avs[h][0:K, psl], bc_sb)
                        if h == 1:
                            (nc.sync if i16 % 2 == 0 else nc.gpsimd).dma_start(
                                out=concat_sb[K:P, pr, cols], in_=dst
                            )

                # ---------------- startup -----------------------------
                warm_ps = psa.tile([P, 512], F32, tag="mm", bufs=2, name="warm")

                def warm(n):
                    for _ in range(n):
                        nc.tensor.matmul(
                            warm_ps, warm_sb[:, 0:P], warm_sb,
                            start=True, stop=True,
                        )

                warm(6)
                # p0/p1 projections + first V tiles emitted directly in
                # DMA-arrival order; the rest become deadline-tagged fill.
                for ops in (
                    proj_unit_ops(0, 0, "q"),
                    proj_unit_ops(0, 0, "k"),
                    proj_unit_ops(1, 0, "q"),
                    proj_unit_ops(1, 0, "k"),
                    v_group_ops(0),
                    v_group_ops(1),
                    v_group_ops(2),
                    v_group_ops(3),
                ):
                    for fn, cost in ops:
                        fn()
                        clock["pe"] += cost

                # fill inventory, deadline = segment index (tt*4+pr)
                fill_extend(2, proj_unit_ops(2, 0, "q"))
                fill_extend(2, proj_unit_ops(2, 0, "k"))
                fill_extend(3, proj_unit_ops(3, 0, "q"))
                fill_extend(3, proj_unit_ops(3, 0, "k"))
                for tt in range(1, NT):
                    for s in range(4 * tt, 4 * tt + 4):
                        fill_extend(4 * tt, v_group_ops(s))
                    for pr in range(NPAIR):
                        fill_extend(4 * tt + pr, proj_unit_ops(pr, tt, "q"))
                        fill_extend(4 * tt + pr, proj_unit_ops(pr, tt, "k"))

                # ---------------- main tt-major loop ------------------
                for tt in range(NT):
                    n_s = 4 * tt + 4
                    for pr in range(NPAIR):
                        seg = 4 * tt + pr
                        flush_due(seg)
                        avs = [
                            psa.tile(
                                [K + 1, 512], F32, tag="av", bufs=2,
                                name=f"av{pr}_{tt}_{h2}",
                            )
                            for h2 in range(2)
                        ]
                        prev = None
                        for si in range(n_s):
                            # score pair needs the st slot freed by the
                            # exp two score-pairs back (global rotation)
                            pace_to(exp_hist[-2] + 100)
                            ex, nv, off = score_exp(pr, tt, si)
                            if prev is not None:
                                pace_to(
                                    sc_done[(pr, tt, si - 1)]
                                    + 100
                                )
                                av_pair(pr, tt, si - 1, *prev, n_s)
                            prev = (ex, nv, off)
                        pace_to(sc_done[(pr, tt, n_s - 1)] + 100)
                        av_pair(pr, tt, n_s - 1, *prev, n_s)
                        if pr == NPAIR - 1:
                            for i16 in range(4):
                                slice_normalize(pr, tt, avs, i16)
                                t16 = 4 * tt + i16
                                fill_extend(99, op_group_ops(t16, 0))
                                fill_extend(99, op_group_ops(t16, 1))
                        else:
                            direct_normalize(pr, tt, avs)

                # ---------------- flush -------------------------------
                flush_mode[0] = True
                while fillq:
                    pop_fill()
    _fuse_score_ldweights(nc)
    nc.compile()
    return nc


def _fuse_score_ldweights(nc):
    """Merge each score pair's two 64-row LDWEIGHTS into one 128-row load.

    The post-Tile IR carries [Ldw(h0 64p), MM(0,0), Ldw(h1 64p), MM(64,0)]
    per key tile. With two LDWs the PE stalls ~100ns on each side of the
    pair (single background weight buffer). One 128-row LDW loads both
    heads' K slice at once; the row-tiled matmuls then address their own
    row groups of the already-loaded array.
    """
    fn = list(nc.m.functions)[0]
    fused = 0
    for blk in fn.blocks:
        insts = blk.instructions
        # pattern-match on the PE-engine subsequence: other engines'
        # instructions interleave freely in the block list
        pe = [
            (i, x)
            for i, x in enumerate(insts)
            if type(x).__name__ in ("InstLdweights", "InstMatmult")
        ]
        drop = []
        for k in range(len(pe) - 3):
            (_, a), (_, b), (ic, c), (_, d) = pe[k], pe[k + 1], pe[k + 2], pe[k + 3]
            if not (
                type(a).__name__ == "InstLdweights"
                and type(b).__name__ == "InstMatmult"
                and type(c).__name__ == "InstLdweights"
                and type(d).__name__ == "InstMatmult"
            ):
                continue
            if not (
                tuple(b.tile_size or ()) == (64, 128)
                and tuple(b.tile_position or ()) == (0, 0)
                and tuple(d.tile_size or ()) == (64, 128)
                and tuple(d.tile_position or ()) == (64, 0)
            ):
                continue
            apA, apC = a.ins[0], c.ins[0]
            pa, pc = list(apA.ap), list(apC.ap)
            if not (
                len(pa) == 2
                and pa[0][1] == 64
                and pc[0][1] == 64
                and pa[0][0] == pc[0][0]
                and pa[1] == pc[1]
                and apC.offset == apA.offset + 64 * pa[0][0]
                and c.sync_info is None
            ):
                continue
            apA.ap = [[pa[0][0], 128], pa[1]]
            if tuple(a.tile_size or ()) == (64, 128):
                a.tile_size = (128, 128)
            a.merge_dependencies_from(c)
            drop.append(ic)
            fused += 1
        for j in sorted(drop, reverse=True):
            del insts[j]
    assert fused > 0, "score LDW fusion matched nothing"


def shard_inputs(X, Wq, Wk, Wv, Wo):
    """Host-side shard prep: core c handles batch c//2, head group c%2."""
    in_maps = []
    for c in range(8):
        b, g = c // 2, c % 2
        heads = range(g * HG, (g + 1) * HG)
        wq = np.stack(
            [
                np.concatenate([Wq[g * HG + 2 * p], Wq[g * HG + 2 * p + 1]], axis=1)
                for p in range(NPAIR)
            ]
        )
        wk = np.stack(
            [
                np.concatenate([Wk[g * HG + 2 * p], Wk[g * HG + 2 * p + 1]], axis=1)
                for p in range(NPAIR)
            ]
        )
        wv = np.concatenate([Wv[h] for h in heads], axis=1)
        wo = Wo[:, g * 512 : (g + 1) * 512].T
        in_maps.append(
            {
                "xt": np.ascontiguousarray(X[b].T).astype(np.float16),
                "wq": np.ascontiguousarray(wq).astype(np.float16),
                "wk": np.ascontiguousarray(wk).astype(np.float16),
                "wv": np.ascontiguousarray(wv).astype(np.float16),
                "wo": np.ascontiguousarray(wo).astype(np.float16),
            }
        )
    return in_maps


_MODULE = None


def _get_module():
    global _MODULE
    if _MODULE is None:
        _MODULE = build_module()
    return _MODULE


def kernel(X, Wq, Wk, Wv, Wo, bo, _want_results=None):
    from concourse.bass_utils import run_bass_kernel_spmd

    nc = _get_module()
    in_maps = shard_inputs(
        np.asarray(X), np.asarray(Wq), np.asarray(Wk), np.asarray(Wv), np.asarray(Wo)
    )
    res = run_bass_kernel_spmd(nc, in_maps, core_ids=list(range(8)))
    if _want_results is not None:
        _want_results.append(res)
    out = np.empty((B, T, H * K), dtype=np.float32)
    bo = np.asarray(bo, dtype=np.float32)
    for b in range(B):
        out[b] = (
            res.results[2 * b]["out"].astype(np.float32)
            + res.results[2 * b + 1]["out"].astype(np.float32)
            + bo
        )
    return out


# revision 17
# speedup vs baseline: 1.0632x; 1.0358x over previous
"""Multi-head causal self-attention on 8 Trainium2 NeuronCores.

Problem: X[4,2048,1024], per-head Wq/Wk/Wv[16,1024,64], Wo[1024,1024], bo[1024].
    out = OutProj(concat_heads(softmax_causal(Q K^T / 8) V))

Sharding: 8 cores = 4 batches x 2 head-groups (8 heads each). Each core
computes its batch's attention for its 8 heads plus the partial output
projection over its 512 concat features; host sums the two partials per
batch and adds the bias.

Per-core kernel (matmul operands in fp16 — 1 col/cycle on TensorE with
fp32 PSUM accumulation; softmax runs in the transposed
"feature-on-partition" space so its reduction lands on the free dim):
  qT/kT per head-pair  [128, T]  = Wpair^T  x  X^T
  v    per s-tile      [128, 8*65] = X^T^T  x  Wv_all (65th col set to 1)
  ST block [s=128, t=512] = kT_slice^T @ qT_slice   (row-packed head pairs:
     the two 64-row tiles share one fused LDWEIGHTS and stream their
     moving operands CONCURRENTLY — disjoint SBUF partitions + disjoint
     PSUM banks — so a score pair costs ~nv cycles, not 2*nv)
  expST = exp(ST/8) (ScalarE), causal-masked via tri multiply
  avT [65, 512] += [V|1]^T @ expST   -> rows 0:64 = (A@V)^T, row 64 = sums
  normalize via 1/sums broadcast and write concatT
  partial = concatT^T @ WoST  (accumulated over 4 feature chunks)

Schedule (v2): tt-MAJOR — for each 512-wide query tile tt, all four head
pairs run their attention segment back-to-back.  All pairs' Q/K live in
SBUF simultaneously, so the output projection for query block tt unlocks
as soon as phase tt completes (25/50/75/100% marks) instead of piling
into the last quarter.  Fill work (later-phase projections, V tail,
out-proj groups) is interleaved into the attention stream under a
simple clock model of PE vs ScalarE so the in-order PE queue never
head-of-line blocks on an exp that hasn't fired: per si step the AV
matmuls trail the score pair by one step, and filler is popped until
the PE clock catches the predicted exp completion.
"""

import os
import sys

for _p in ("/opt/trn_rl_repo", "/root/.axon_site/_ro/trn_rl_repo"):
    if os.path.isdir(_p) and _p not in sys.path:
        sys.path.append(_p)

import numpy as np

import concourse.mybir as mybir
import concourse.tile as tile
from concourse import bacc

B, T, D, H, K = 4, 2048, 1024, 16, 64
HG = 8          # heads per core
NPAIR = 4       # head pairs per core
P = 128
DC = D // P     # 8 contraction chunks for the projections
NS = T // P     # 16 key tiles
NT = T // 512   # 4 query tiles of 512
F32 = mybir.dt.float32
F16 = mybir.dt.float16

# clock-model constants (ns), calibrated from the v1 trace
MM_NS = 216.0 / 512.0      # per streamed column, 512-col mm ~216ns cadence
PAIR_FIX = 100.0           # extra fixed cost of a score-pair issue
EXP_COL = 0.87             # ScalarE ns per column
EXP_FIX = 260.0            # ScalarE per-activation overhead
EXP_LAG = 220.0            # sem propagation mm-done -> exp start


def build_module():
    nc = bacc.Bacc("TRN2")
    XT = nc.dram_tensor("xt", [D, T], F16, kind="ExternalInput").ap()
    WQ = nc.dram_tensor("wq", [NPAIR, D, P], F16, kind="ExternalInput").ap()
    WK = nc.dram_tensor("wk", [NPAIR, D, P], F16, kind="ExternalInput").ap()
    WV = nc.dram_tensor("wv", [D, HG * K], F16, kind="ExternalInput").ap()
    WO = nc.dram_tensor("wo", [HG * K, D], F16, kind="ExternalInput").ap()
    OUT = nc.dram_tensor("out", [T, D], F16, kind="ExternalOutput").ap()

    with tile.TileContext(nc) as tc:
        with tc.tile_pool(name="persist", bufs=1) as pp:
            xt_sb = pp.tile([P, DC, T], F16)            # X^T, 32 KB/partition
            v_sb = pp.tile([P, NS, HG * (K + 1)], F16)  # V + ones col per head
            concat_sb = pp.tile([P, NPAIR, T], F16)     # concat(heads)^T
            tri_sb = pp.tile([P, P], F16)   # causal triangle: 1 where x >= p
            warm_sb = pp.tile([P, 512], F16)
            wo_sb = pp.tile([P, NPAIR, D], F16)
            wv_sb = pp.tile([P, DC, HG * K], F16)
            wq_sb = [pp.tile([P, DC, P], F16, name=f"wq{p}") for p in range(NPAIR)]
            wk_sb = [pp.tile([P, DC, P], F16, name=f"wk{p}") for p in range(NPAIR)]
            q_sb = [pp.tile([P, T], F16, name=f"q{p}") for p in range(NPAIR)]
            k_sb = [pp.tile([P, T], F16, name=f"k{p}") for p in range(NPAIR)]

            xt_r = XT.rearrange("(c p) t -> c p t", p=P)
            xt_p = XT.rearrange("(c p) t -> p c t", p=P)
            wv_p = WV.rearrange("(c p) n -> p c n", p=P)
            wo_p = WO.rearrange("(s p) o -> p s o", p=P)

            # ---- DMA priority emission -------------------------------
            # The sync and gpsimd HW queues sustain ~2x the bandwidth of
            # the scalar queue (measured), so they carry everything
            # needed in the first ~35us in strict need-order; the scalar
            # queue gets only the late-need bytes (Wo, X block 3) and
            # then stays exp-pure.
            # sync: X b0 c0-3, Wq0, Wv[0:4], Wq1-3, Xb1/Xb2 halves
            # gpsimd: X b0 c4-7, Wk0, Wv[4:8], Wk1-3, Xb1/Xb2 halves
            for c in range(4):
                nc.sync.dma_start(out=xt_sb[:, c, 0:512], in_=xt_r[c][:, 0:512])
                nc.gpsimd.dma_start(
                    out=xt_sb[:, c + 4, 0:512], in_=xt_r[c + 4][:, 0:512]
                )
            nc.sync.dma_start(
                out=wq_sb[0], in_=WQ[0].rearrange("(c p) m -> p c m", p=P)
            )
            nc.gpsimd.dma_start(
                out=wk_sb[0], in_=WK[0].rearrange("(c p) m -> p c m", p=P)
            )
            nc.sync.dma_start(out=wv_sb[:, 0:4, :], in_=wv_p[:, 0:4, :])
            nc.gpsimd.dma_start(out=wv_sb[:, 4:8, :], in_=wv_p[:, 4:8, :])
            for pr in range(1, NPAIR):
                nc.sync.dma_start(
                    out=wq_sb[pr], in_=WQ[pr].rearrange("(c p) m -> p c m", p=P)
                )
                nc.gpsimd.dma_start(
                    out=wk_sb[pr], in_=WK[pr].rearrange("(c p) m -> p c m", p=P)
                )
            nc.scalar.dma_start(out=wo_sb[:, 0:2, :], in_=wo_p[:, 0:2, :])
            nc.scalar.dma_start(out=wo_sb[:, 2:4, :], in_=wo_p[:, 2:4, :])
            nc.sync.dma_start(
                out=xt_sb[:, 0:4, 512:1024], in_=xt_p[:, 0:4, 512:1024]
            )
            nc.gpsimd.dma_start(
                out=xt_sb[:, 4:8, 512:1024], in_=xt_p[:, 4:8, 512:1024]
            )
            nc.sync.dma_start(
                out=xt_sb[:, 0:4, 1024:1536], in_=xt_p[:, 0:4, 1024:1536]
            )
            nc.gpsimd.dma_start(
                out=xt_sb[:, 4:8, 1024:1536], in_=xt_p[:, 4:8, 1024:1536]
            )
            nc.scalar.dma_start(
                out=xt_sb[:, 0:4, 1536:2048], in_=xt_p[:, 0:4, 1536:2048]
            )
            nc.scalar.dma_start(
                out=xt_sb[:, 4:8, 1536:2048], in_=xt_p[:, 4:8, 1536:2048]
            )

            nc.vector.memset(warm_sb, 0.0)
            nc.vector.memset(tri_sb, 1.0)
            nc.gpsimd.affine_select(
                out=tri_sb,
                in_=tri_sb,
                compare_op=mybir.AluOpType.is_ge,
                fill=0.0,
                base=0,
                channel_multiplier=-1,
                pattern=[[1, P]],
            )
            # ones column (index 64 of each head's 65-wide slot)
            v_slots = v_sb.rearrange("p s (h x) -> p s h x", x=K + 1)
            nc.vector.memset(v_slots[:, :, :, K : K + 1], 1.0)

            with (
                tc.tile_pool(name="attn", bufs=1) as ap_,
                tc.tile_pool(name="psa", bufs=1, space="PSUM") as psa,
            ):
                # ---------------- op builders -------------------------
                def v_group_ops(s):
                    """V projection for one key tile: 8 mms + 1 cast."""
                    holder = {}

                    def mm(c):
                        def f():
                            if "ps" not in holder:
                                holder["ps"] = psa.tile(
                                    [P, HG * K], F32, tag="mm", bufs=2,
                                    name=f"vps{s}",
                                )
                            nc.tensor.matmul(
                                holder["ps"],
                                xt_sb[:, c, s * P : (s + 1) * P],
                                wv_sb[:, c, :],
                                start=(c == 0),
                                stop=(c == DC - 1),
                            )
                        return f

                    def fin():
                        nc.vector.tensor_copy(
                            v_slots[:, s, :, 0:K],
                            holder["ps"].rearrange("p (h k) -> p h k", k=K),
                        )

                    return [(mm(c), MM_NS * 512) for c in range(DC)] + [(fin, 0.0)]

                def proj_unit_ops(pr, tt, which):
                    """Q or K projection for (pair, query tile): 8 mms+cast."""
                    w_sb = wq_sb[pr] if which == "q" else wk_sb[pr]
                    dst = q_sb[pr] if which == "q" else k_sb[pr]
                    holder = {}

                    def mm(c):
                        def f():
                            if "ps" not in holder:
                                holder["ps"] = psa.tile(
                                    [P, 512], F32, tag="mm", bufs=2,
                                    name=f"{which}ps{pr}_{tt}",
                                )
                            nc.tensor.matmul(
                                holder["ps"],
                                w_sb[:, c, :],
                                xt_sb[:, c, tt * 512 : (tt + 1) * 512],
                                start=(c == 0),
                                stop=(c == DC - 1),
                            )
                        return f

                    def fin():
                        nc.vector.tensor_copy(
                            dst[:, tt * 512 : (tt + 1) * 512], holder["ps"]
                        )

                    return [(mm(c), MM_NS * 512) for c in range(DC)] + [(fin, 0.0)]

                out_q = [nc.gpsimd, nc.sync]
                out_qi = [0]
                flush_mode = [False]

                def op_group_ops(t16, oc):
                    """Output-projection group for one [128 t, 512 oc] tile."""
                    holder = {}

                    def mm(s4):
                        def f():
                            if "ps" not in holder:
                                holder["ps"] = psa.tile(
                                    [P, 512], F32, tag="mm", bufs=2,
                                    name=f"ops{t16}_{oc}",
                                )
                            nc.tensor.matmul(
                                holder["ps"],
                                concat_sb[:, s4, t16 * P : (t16 + 1) * P],
                                wo_sb[:, s4, oc * 512 : (oc + 1) * 512],
                                start=(s4 == 0),
                                stop=(s4 == NPAIR - 1),
                            )
                        return f

                    def fin():
                        st_o = ap_.tile(
                            [P, 512], F16, tag="outst", bufs=6,
                            name=f"ost{t16}_{oc}",
                        )
                        if flush_mode[0] and (t16 + oc) % 2 == 0:
                            nc.scalar.copy(st_o, holder["ps"])
                        else:
                            nc.vector.tensor_copy(st_o, holder["ps"])
                        eng = out_q[out_qi[0] % len(out_q)]
                        out_qi[0] += 1
                        eng.dma_start(
                            out=OUT[
                                t16 * P : (t16 + 1) * P,
                                oc * 512 : (oc + 1) * 512,
                            ],
                            in_=st_o,
                        )

                    return [(mm(s4), MM_NS * 512) for s4 in range(NPAIR)] + [
                        (fin, 0.0)
                    ]

                # ---------------- fill queue --------------------------
                # entries [deadline_seg, cost_ns, ready_ns, kind, fn]
                # kind "pre": must run before the deadline segment's
                # scores (projections); kind "av": before its first AV
                # (V tiles); kind "op": no deadline (out-proj).
                fillq = []

                def fill_extend(deadline, ops, ready=0.0, kind="pre"):
                    for fn, cost in ops:
                        fillq.append([deadline, cost, ready, kind, fn])

                clock = {"pe": 11000.0, "sc": 11000.0}
                sc_done = {}
                exp_hist = [0.0, 0.0]  # completion of last two exps (global)

                def pop_fill():
                    """Emit the first fill op whose data has landed."""
                    for idx in range(min(len(fillq), 24)):
                        if fillq[idx][2] <= clock["pe"]:
                            _, cost, _, _, fn = fillq.pop(idx)
                            fn()
                            clock["pe"] += cost
                            return True
                    return False

                def pace_to(target):
                    while clock["pe"] < target:
                        if not pop_fill():
                            clock["pe"] = target
                            break

                def flush_due(seg, kinds=("pre",)):
                    idx = 0
                    while idx < len(fillq):
                        dl, cost, _, kind, fn = fillq[idx]
                        if dl <= seg and kind in kinds:
                            fillq.pop(idx)
                            fn()
                            clock["pe"] += cost
                        else:
                            idx += 1

                # ---------------- attention pieces --------------------
                def score_exp(pr, tt, si):
                    m = si - 4 * tt
                    off = max(m, 0) * P
                    nv = 512 - off
                    st = psa.tile([P, 2, 512], F32, tag="stw", bufs=2)
                    ex = ap_.tile(
                        [P, 2, 512], F16, tag="exp", bufs=8,
                        name=f"exp{pr}_{tt}_{si}",
                    )
                    for h in range(2):
                        lo, hi = h * K, (h + 1) * K
                        nc.tensor.matmul(
                            st[:, h, 0:nv],
                            k_sb[pr][lo:hi, si * P : (si + 1) * P],
                            q_sb[pr][lo:hi, tt * 512 + off : (tt + 1) * 512],
                            start=True,
                            stop=True,
                            tile_position=(lo, 0),
                        )
                    clock["pe"] += MM_NS * nv + PAIR_FIX
                    nc.scalar.activation(
                        ex[:, :, 0:nv], st[:, :, 0:nv],
                        mybir.ActivationFunctionType.Exp,
                        scale=0.125,
                    )
                    start = max(clock["sc"], clock["pe"] + EXP_LAG)
                    clock["sc"] = start + 2 * nv * EXP_COL + EXP_FIX
                    sc_done[(pr, tt, si)] = clock["sc"]
                    exp_hist.append(clock["sc"])
                    if m >= 0:  # mask both heads' leading triangles
                        nc.vector.tensor_mul(
                            ex[:, :, 0:P],
                            ex[:, :, 0:P],
                            tri_sb.unsqueeze(1).broadcast_to([P, 2, P]),
                        )
                    return ex, nv, off

                def av_pair(pr, tt, si, ex, nv, off, n_s):
                    for h in range(2):
                        slot = (2 * pr + h) * (K + 1)
                        nc.tensor.matmul(
                            avs[h][:, off:512],
                            v_sb[:, si, slot : slot + K + 1],
                            ex[:, h, 0:nv],
                            start=(si == 0),
                            stop=(si == n_s - 1),
                        )
                    clock["pe"] += 2 * MM_NS * nv

                def direct_normalize(pr, tt, avs):
                    for h in range(2):
                        cols = slice(tt * 512, (tt + 1) * 512)
                        sums = ap_.tile([1, 512], F32, tag="sums", bufs=6)
                        nc.vector.tensor_copy(sums, avs[h][K : K + 1, :])
                        recip = ap_.tile([1, 512], F32, tag="recip", bufs=6)
                        nc.vector.reciprocal_approx_fast(recip, sums)
                        bc_sb = ap_.tile([K, 512], F32, tag="bc_sb", bufs=6)
                        nc.gpsimd.partition_broadcast(bc_sb, recip)
                        if h == 0:
                            dst = concat_sb[0:K, pr, cols]
                        else:
                            dst = ap_.tile([K, 512], F16, tag="tmpb", bufs=6)
                        nc.vector.tensor_mul(dst, avs[h][0:K, :], bc_sb)
                        if h == 1:
                            nc.gpsimd.dma_start(
                                out=concat_sb[K:P, pr, cols], in_=dst
                            )

                def slice_normalize(pr, tt, avs, i16):
                    """128-col slice normalize for the phase's last pair,
                    so each out-proj group unlocks as early as possible."""
                    cols_lo = i16 * P
                    for h in range(2):
                        cols = slice(tt * 512 + cols_lo, tt * 512 + cols_lo + P)
                        psl = slice(cols_lo, cols_lo + P)
                        sums = ap_.tile([1, P], F32, tag="sums", bufs=6)
                        nc.vector.tensor_copy(sums, avs[h][K : K + 1, psl])
                        recip = ap_.tile([1, P], F32, tag="recip", bufs=6)
                        nc.vector.reciprocal_approx_fast(recip, sums)
                        bc_sb = ap_.tile([K, P], F32, tag="bc_sb", bufs=6)
                        nc.gpsimd.partition_broadcast(bc_sb, recip)
                        if h == 0:
                            dst = concat_sb[0:K, pr, cols]
                        else:
                            dst = ap_.tile([K, P], F16, tag="tmpb", bufs=6)
                        nc.vector.tensor_mul(dst, avs[h][0:K, psl], bc_sb)
                        if h == 1:
                            (nc.gpsimd if i16 % 2 == 0 else nc.sync).dma_start(
                                out=concat_sb[K:P, pr, cols], in_=dst
                            )

                # ---------------- startup -----------------------------
                warm_ps = psa.tile([P, 512], F32, tag="mm", bufs=2, name="warm")

                def warm(n):
                    for _ in range(n):
                        nc.tensor.matmul(
                            warm_ps, warm_sb[:, 0:P], warm_sb,
                            start=True, stop=True,
                        )

                warm(6)
                # p0's projections emitted directly in DMA-arrival order
                # (chunks alternate between the two fast queues); scores
                # for (p0, tt0) then start while everything else streams.
                for which in ("q", "k"):
                    ops = proj_unit_ops(0, 0, which)
                    for c in (0, 4, 1, 5, 2, 6, 3, 7):
                        fn, cost = ops[c]
                        fn()
                        clock["pe"] += cost
                    ops[DC][0]()  # cast

                # fill inventory, deadline = segment index (tt*4+pr).
                # ready_ns = rough DMA landing estimate for the op's data.
                for s in range(4):
                    fill_extend(0, v_group_ops(s), ready=19000, kind="av")
                for pr in range(1, NPAIR):
                    rdy = 20000 + 2000 * pr
                    fill_extend(pr, proj_unit_ops(pr, 0, "q"), ready=rdy)
                    fill_extend(pr, proj_unit_ops(pr, 0, "k"), ready=rdy)
                for tt in range(1, NT):
                    xt_rdy = [0, 31000, 38000, 40000][tt]
                    for s in range(4 * tt, 4 * tt + 4):
                        fill_extend(
                            4 * tt, v_group_ops(s), ready=xt_rdy, kind="av"
                        )
                    for pr in range(NPAIR):
                        fill_extend(
                            4 * tt + pr, proj_unit_ops(pr, tt, "q"), ready=xt_rdy
                        )
                        fill_extend(
                            4 * tt + pr, proj_unit_ops(pr, tt, "k"), ready=xt_rdy
                        )

                # ---------------- main tt-major loop ------------------
                for tt in range(NT):
                    n_s = 4 * tt + 4
                    for pr in range(NPAIR):
                        seg = 4 * tt + pr
                        flush_due(seg)
                        avs = [
                            psa.tile(
                                [K + 1, 512], F32, tag="av", bufs=2,
                                name=f"av{pr}_{tt}_{h2}",
                            )
                            for h2 in range(2)
                        ]
                        # AV trails the score stream by 2 steps so an AV
                        # at the PE queue head never waits on its exp.
                        pend = {}
                        for si in range(n_s):
                            # score pair needs the st slot freed by the
                            # exp two score-pairs back (global rotation)
                            pace_to(exp_hist[-2] + 100)
                            pend[si] = score_exp(pr, tt, si)
                            if si == 1:
                                # V tiles must be in the stream before
                                # the segment's first AV
                                flush_due(seg, kinds=("pre", "av"))
                            if si >= 2:
                                pace_to(sc_done[(pr, tt, si - 2)] + 80)
                                av_pair(pr, tt, si - 2, *pend.pop(si - 2), n_s)
                        for sj in (n_s - 2, n_s - 1):
                            pace_to(sc_done[(pr, tt, sj)] + 80)
                            av_pair(pr, tt, sj, *pend.pop(sj), n_s)
                        if pr == NPAIR - 1:
                            for i16 in range(4):
                                slice_normalize(pr, tt, avs, i16)
                                t16 = 4 * tt + i16
                                fill_extend(99, op_group_ops(t16, 0))
                                fill_extend(99, op_group_ops(t16, 1))
                        else:
                            direct_normalize(pr, tt, avs)

                # ---------------- flush -------------------------------
                flush_mode[0] = True
                while fillq:
                    pop_fill()
    _fuse_score_ldweights(nc)
    nc.compile()
    return nc


def _fuse_score_ldweights(nc):
    """Merge each score pair's two 64-row LDWEIGHTS into one 128-row load.

    The post-Tile IR carries [Ldw(h0 64p), MM(0,0), Ldw(h1 64p), MM(64,0)]
    per key tile. With two LDWs the PE stalls ~100ns on each side of the
    pair (single background weight buffer). One 128-row LDW loads both
    heads' K slice at once; the row-tiled matmuls then address their own
    row groups of the already-loaded array.
    """
    fn = list(nc.m.functions)[0]
    fused = 0
    for blk in fn.blocks:
        insts = blk.instructions
        # pattern-match on the PE-engine subsequence: other engines'
        # instructions interleave freely in the block list
        pe = [
            (i, x)
            for i, x in enumerate(insts)
            if type(x).__name__ in ("InstLdweights", "InstMatmult")
        ]
        drop = []
        for k in range(len(pe) - 3):
            (_, a), (_, b), (ic, c), (_, d) = pe[k], pe[k + 1], pe[k + 2], pe[k + 3]
            if not (
                type(a).__name__ == "InstLdweights"
                and type(b).__name__ == "InstMatmult"
                and type(c).__name__ == "InstLdweights"
                and type(d).__name__ == "InstMatmult"
            ):
                continue
            if not (
                tuple(b.tile_size or ()) == (64, 128)
                and tuple(b.tile_position or ()) == (0, 0)
                and tuple(d.tile_size or ()) == (64, 128)
                and tuple(d.tile_position or ()) == (64, 0)
            ):
                continue
            apA, apC = a.ins[0], c.ins[0]
            pa, pc = list(apA.ap), list(apC.ap)
            if not (
                len(pa) == 2
                and pa[0][1] == 64
                and pc[0][1] == 64
                and pa[0][0] == pc[0][0]
                and pa[1] == pc[1]
                and apC.offset == apA.offset + 64 * pa[0][0]
                and c.sync_info is None
            ):
                continue
            apA.ap = [[pa[0][0], 128], pa[1]]
            if tuple(a.tile_size or ()) == (64, 128):
                a.tile_size = (128, 128)
            a.merge_dependencies_from(c)
            drop.append(ic)
            fused += 1
        for j in sorted(drop, reverse=True):
            del insts[j]
    assert fused > 0, "score LDW fusion matched nothing"


def shard_inputs(X, Wq, Wk, Wv, Wo):
    """Host-side shard prep: core c handles batch c//2, head group c%2."""
    in_maps = []
    for c in range(8):
        b, g = c // 2, c % 2
        heads = range(g * HG, (g + 1) * HG)
        wq = np.stack(
            [
                np.concatenate([Wq[g * HG + 2 * p], Wq[g * HG + 2 * p + 1]], axis=1)
                for p in range(NPAIR)
            ]
        )
        wk = np.stack(
            [
                np.concatenate([Wk[g * HG + 2 * p], Wk[g * HG + 2 * p + 1]], axis=1)
                for p in range(NPAIR)
            ]
        )
        wv = np.concatenate([Wv[h] for h in heads], axis=1)
        wo = Wo[:, g * 512 : (g + 1) * 512].T
        in_maps.append(
            {
                "xt": np.ascontiguousarray(X[b].T).astype(np.float16),
                "wq": np.ascontiguousarray(wq).astype(np.float16),
                "wk": np.ascontiguousarray(wk).astype(np.float16),
                "wv": np.ascontiguousarray(wv).astype(np.float16),
                "wo": np.ascontiguousarray(wo).astype(np.float16),
            }
        )
    return in_maps


_MODULE = None


def _get_module():
    global _MODULE
    if _MODULE is None:
        _MODULE = build_module()
    return _MODULE


def kernel(X, Wq, Wk, Wv, Wo, bo, _want_results=None):
    from concourse.bass_utils import run_bass_kernel_spmd

    nc = _get_module()
    in_maps = shard_inputs(
        np.asarray(X), np.asarray(Wq), np.asarray(Wk), np.asarray(Wv), np.asarray(Wo)
    )
    res = run_bass_kernel_spmd(nc, in_maps, core_ids=list(range(8)))
    if _want_results is not None:
        _want_results.append(res)
    out = np.empty((B, T, H * K), dtype=np.float32)
    bo = np.asarray(bo, dtype=np.float32)
    for b in range(B):
        out[b] = (
            res.results[2 * b]["out"].astype(np.float32)
            + res.results[2 * b + 1]["out"].astype(np.float32)
            + bo
        )
    return out


# revision 20
# speedup vs baseline: 1.0684x; 1.0049x over previous
"""Multi-head causal self-attention on 8 Trainium2 NeuronCores.

Problem: X[4,2048,1024], per-head Wq/Wk/Wv[16,1024,64], Wo[1024,1024], bo[1024].
    out = OutProj(concat_heads(softmax_causal(Q K^T / 8) V))

Sharding: 8 cores = 4 batches x 2 head-groups (8 heads each). Each core
computes its batch's attention for its 8 heads plus the partial output
projection over its 512 concat features; host sums the two partials per
batch and adds the bias.

Per-core kernel (matmul operands in fp16 — 1 col/cycle on TensorE with
fp32 PSUM accumulation; softmax runs in the transposed
"feature-on-partition" space so its reduction lands on the free dim):
  qT/kT per head-pair  [128, T]  = Wpair^T  x  X^T
  v    per s-tile      [128, 8*65] = X^T^T  x  Wv_all (65th col set to 1)
  ST block [s=128, t=512] = kT_slice^T @ qT_slice   (row-packed head pairs:
     the two 64-row tiles share one fused LDWEIGHTS and stream their
     moving operands CONCURRENTLY — disjoint SBUF partitions + disjoint
     PSUM banks — so a score pair costs ~nv cycles, not 2*nv)
  expST = exp(ST/8) (ScalarE), causal-masked via tri multiply
  avT [65, 512] += [V|1]^T @ expST   -> rows 0:64 = (A@V)^T, row 64 = sums
  normalize via 1/sums broadcast and write concatT
  partial = concatT^T @ WoST  (accumulated over 4 feature chunks)

Schedule (v2): tt-MAJOR — for each 512-wide query tile tt, all four head
pairs run their attention segment back-to-back.  All pairs' Q/K live in
SBUF simultaneously, so the output projection for query block tt unlocks
as soon as phase tt completes (25/50/75/100% marks) instead of piling
into the last quarter.  Fill work (later-phase projections, V tail,
out-proj groups) is interleaved into the attention stream under a
simple clock model of PE vs ScalarE so the in-order PE queue never
head-of-line blocks on an exp that hasn't fired: per si step the AV
matmuls trail the score pair by one step, and filler is popped until
the PE clock catches the predicted exp completion.
"""

import os
import sys

for _p in ("/opt/trn_rl_repo", "/root/.axon_site/_ro/trn_rl_repo"):
    if os.path.isdir(_p) and _p not in sys.path:
        sys.path.append(_p)

import numpy as np

import concourse.mybir as mybir
import concourse.tile as tile
from concourse import bacc

B, T, D, H, K = 4, 2048, 1024, 16, 64
HG = 8          # heads per core
NPAIR = 4       # head pairs per core
P = 128
DC = D // P     # 8 contraction chunks for the projections
NS = T // P     # 16 key tiles
NT = T // 512   # 4 query tiles of 512
F32 = mybir.dt.float32
F16 = mybir.dt.float16

# clock-model constants (ns), calibrated from the v1 trace
MM_NS = 216.0 / 512.0      # per streamed column, 512-col mm ~216ns cadence
PAIR_FIX = 100.0           # extra fixed cost of a score-pair issue
EXP_COL = 0.87             # ScalarE ns per column
EXP_FIX = 260.0            # ScalarE per-activation overhead
EXP_LAG = 220.0            # sem propagation mm-done -> exp start


def build_module():
    nc = bacc.Bacc("TRN2")
    XT = nc.dram_tensor("xt", [D, T], F16, kind="ExternalInput").ap()
    WQ = nc.dram_tensor("wq", [NPAIR, D, P], F16, kind="ExternalInput").ap()
    WK = nc.dram_tensor("wk", [NPAIR, D, P], F16, kind="ExternalInput").ap()
    WV = nc.dram_tensor("wv", [D, HG * K], F16, kind="ExternalInput").ap()
    WO = nc.dram_tensor("wo", [HG * K, D], F16, kind="ExternalInput").ap()
    OUT = nc.dram_tensor("out", [T, D], F16, kind="ExternalOutput").ap()

    with tile.TileContext(nc) as tc:
        with tc.tile_pool(name="persist", bufs=1) as pp:
            xt_sb = pp.tile([P, DC, T], F16)            # X^T, 32 KB/partition
            v_sb = pp.tile([P, NS, HG * (K + 1)], F16)  # V + ones col per head
            concat_sb = pp.tile([P, NPAIR, T], F16)     # concat(heads)^T
            tri_sb = pp.tile([P, P], F16)   # causal triangle: 1 where x >= p
            warm_sb = pp.tile([P, 512], F16)
            wo_sb = pp.tile([P, NPAIR, D], F16)
            wv_sb = pp.tile([P, DC, HG * K], F16)
            wq_sb = [pp.tile([P, DC, P], F16, name=f"wq{p}") for p in range(NPAIR)]
            wk_sb = [pp.tile([P, DC, P], F16, name=f"wk{p}") for p in range(NPAIR)]
            q_sb = [pp.tile([P, T], F16, name=f"q{p}") for p in range(NPAIR)]
            k_sb = [pp.tile([P, T], F16, name=f"k{p}") for p in range(NPAIR)]

            xt_r = XT.rearrange("(c p) t -> c p t", p=P)
            xt_p = XT.rearrange("(c p) t -> p c t", p=P)
            wv_p = WV.rearrange("(c p) n -> p c n", p=P)
            wo_p = WO.rearrange("(s p) o -> p s o", p=P)

            # ---- DMA priority emission -------------------------------
            # Uniform per-chunk 2D transfers (contiguous per-partition
            # source rows) in strict need-order per queue — mixed sizes
            # and 3D patterns make transfer completion and the trigger
            # instructions (semaphore-pool reuse) unpredictable.
            # sync/gpsimd: X b0 split even/odd, Wq0/Wk0, Wq1/Wk1, Wv
            # split, Wq2/Wk2, Wq3/Wk3, X b1/b2 halves.  scalar: only
            # late-need bytes (Wo, X b3) so it stays exp-pure after ~15us.
            for c in range(0, DC, 2):
                nc.sync.dma_start(out=xt_sb[:, c, 0:512], in_=xt_r[c][:, 0:512])
                nc.gpsimd.dma_start(
                    out=xt_sb[:, c + 1, 0:512], in_=xt_r[c + 1][:, 0:512]
                )
            nc.sync.dma_start(
                out=wq_sb[0], in_=WQ[0].rearrange("(c p) m -> p c m", p=P)
            )
            nc.gpsimd.dma_start(
                out=wk_sb[0], in_=WK[0].rearrange("(c p) m -> p c m", p=P)
            )
            nc.sync.dma_start(
                out=wq_sb[1], in_=WQ[1].rearrange("(c p) m -> p c m", p=P)
            )
            nc.gpsimd.dma_start(
                out=wk_sb[1], in_=WK[1].rearrange("(c p) m -> p c m", p=P)
            )
            for c in range(0, DC, 2):
                nc.sync.dma_start(out=wv_sb[:, c, :], in_=wv_p[:, c, :])
                nc.gpsimd.dma_start(
                    out=wv_sb[:, c + 1, :], in_=wv_p[:, c + 1, :]
                )
            for pr in (2, 3):
                nc.sync.dma_start(
                    out=wq_sb[pr], in_=WQ[pr].rearrange("(c p) m -> p c m", p=P)
                )
                nc.gpsimd.dma_start(
                    out=wk_sb[pr], in_=WK[pr].rearrange("(c p) m -> p c m", p=P)
                )
            nc.scalar.dma_start(out=wo_sb[:, 0:2, :], in_=wo_p[:, 0:2, :])
            nc.scalar.dma_start(out=wo_sb[:, 2:4, :], in_=wo_p[:, 2:4, :])
            for c in range(DC):
                (nc.sync if c % 2 == 0 else nc.gpsimd).dma_start(
                    out=xt_sb[:, c, 512:1024], in_=xt_r[c][:, 512:1024]
                )
            for c in range(DC):
                nc.scalar.dma_start(
                    out=xt_sb[:, c, 1536:2048], in_=xt_r[c][:, 1536:2048]
                )
            for c in range(DC):
                (nc.sync if c % 2 == 0 else nc.gpsimd).dma_start(
                    out=xt_sb[:, c, 1024:1536], in_=xt_r[c][:, 1024:1536]
                )

            nc.vector.memset(warm_sb, 0.0)
            nc.vector.memset(tri_sb, 1.0)
            nc.gpsimd.affine_select(
                out=tri_sb,
                in_=tri_sb,
                compare_op=mybir.AluOpType.is_ge,
                fill=0.0,
                base=0,
                channel_multiplier=-1,
                pattern=[[1, P]],
            )
            # ones column (index 64 of each head's 65-wide slot)
            v_slots = v_sb.rearrange("p s (h x) -> p s h x", x=K + 1)
            nc.vector.memset(v_slots[:, :, :, K : K + 1], 1.0)

            with (
                tc.tile_pool(name="attn", bufs=1) as ap_,
                tc.tile_pool(name="psa", bufs=1, space="PSUM") as psa,
            ):
                # ---------------- op builders -------------------------
                def v_group_ops(s):
                    """V projection for one key tile: 8 mms + 1 cast."""
                    holder = {}

                    def mm(c):
                        def f():
                            if "ps" not in holder:
                                holder["ps"] = psa.tile(
                                    [P, HG * K], F32, tag="mm", bufs=2,
                                    name=f"vps{s}",
                                )
                            nc.tensor.matmul(
                                holder["ps"],
                                xt_sb[:, c, s * P : (s + 1) * P],
                                wv_sb[:, c, :],
                                start=(c == 0),
                                stop=(c == DC - 1),
                            )
                        return f

                    def fin():
                        nc.vector.tensor_copy(
                            v_slots[:, s, :, 0:K],
                            holder["ps"].rearrange("p (h k) -> p h k", k=K),
                        )

                    return [(mm(c), MM_NS * 512) for c in range(DC)] + [(fin, 0.0)]

                def proj_unit_ops(pr, tt, which):
                    """Q or K projection for (pair, query tile): 8 mms+cast."""
                    w_sb = wq_sb[pr] if which == "q" else wk_sb[pr]
                    dst = q_sb[pr] if which == "q" else k_sb[pr]
                    holder = {}

                    def mm(c):
                        def f():
                            if "ps" not in holder:
                                holder["ps"] = psa.tile(
                                    [P, 512], F32, tag="mm", bufs=2,
                                    name=f"{which}ps{pr}_{tt}",
                                )
                            nc.tensor.matmul(
                                holder["ps"],
                                w_sb[:, c, :],
                                xt_sb[:, c, tt * 512 : (tt + 1) * 512],
                                start=(c == 0),
                                stop=(c == DC - 1),
                            )
                        return f

                    def fin():
                        nc.vector.tensor_copy(
                            dst[:, tt * 512 : (tt + 1) * 512], holder["ps"]
                        )

                    return [(mm(c), MM_NS * 512) for c in range(DC)] + [(fin, 0.0)]

                out_q = [nc.gpsimd, nc.sync]
                out_qi = [0]
                flush_mode = [False]

                def op_group_ops(t16, oc):
                    """Output-projection group for one [128 t, 512 oc] tile."""
                    holder = {}

                    def mm(s4):
                        def f():
                            if "ps" not in holder:
                                holder["ps"] = psa.tile(
                                    [P, 512], F32, tag="mm", bufs=2,
                                    name=f"ops{t16}_{oc}",
                                )
                            nc.tensor.matmul(
                                holder["ps"],
                                concat_sb[:, s4, t16 * P : (t16 + 1) * P],
                                wo_sb[:, s4, oc * 512 : (oc + 1) * 512],
                                start=(s4 == 0),
                                stop=(s4 == NPAIR - 1),
                            )
                        return f

                    def fin():
                        st_o = ap_.tile(
                            [P, 512], F16, tag="outst", bufs=6,
                            name=f"ost{t16}_{oc}",
                        )
                        if flush_mode[0] and (t16 + oc) % 2 == 0:
                            nc.scalar.copy(st_o, holder["ps"])
                        else:
                            nc.vector.tensor_copy(st_o, holder["ps"])
                        eng = out_q[out_qi[0] % len(out_q)]
                        out_qi[0] += 1
                        eng.dma_start(
                            out=OUT[
                                t16 * P : (t16 + 1) * P,
                                oc * 512 : (oc + 1) * 512,
                            ],
                            in_=st_o,
                        )

                    return [(mm(s4), MM_NS * 512) for s4 in range(NPAIR)] + [
                        (fin, 0.0)
                    ]

                # ---------------- fill queue --------------------------
                # entries [deadline_seg, cost_ns, ready_ns, kind, fn]
                # kind "pre": must run before the deadline segment's
                # scores (projections); kind "av": before its first AV
                # (V tiles); kind "op": no deadline (out-proj).
                fillq = []

                def fill_extend(deadline, ops, ready=0.0, kind="pre"):
                    for fn, cost in ops:
                        fillq.append([deadline, cost, ready, kind, fn])

                clock = {"pe": 11000.0, "sc": 11000.0}
                sc_done = {}
                exp_hist = [0.0, 0.0]  # completion of last two exps (global)

                def pop_fill():
                    """Emit the first fill op whose data has landed."""
                    for idx in range(min(len(fillq), 24)):
                        if fillq[idx][2] <= clock["pe"]:
                            _, cost, _, _, fn = fillq.pop(idx)
                            fn()
                            clock["pe"] += cost
                            return True
                    return False

                def pace_to(target):
                    while clock["pe"] < target:
                        if not pop_fill():
                            clock["pe"] = target
                            break

                def flush_due(seg, kinds=("pre",)):
                    idx = 0
                    while idx < len(fillq):
                        dl, cost, _, kind, fn = fillq[idx]
                        if dl <= seg and kind in kinds:
                            fillq.pop(idx)
                            fn()
                            clock["pe"] += cost
                        else:
                            idx += 1

                # ---------------- attention pieces --------------------
                def score_exp(pr, tt, si):
                    m = si - 4 * tt
                    off = max(m, 0) * P
                    nv = 512 - off
                    st = psa.tile([P, 2, 512], F32, tag="stw", bufs=2)
                    ex = ap_.tile(
                        [P, 2, 512], F16, tag="exp", bufs=8,
                        name=f"exp{pr}_{tt}_{si}",
                    )
                    for h in range(2):
                        lo, hi = h * K, (h + 1) * K
                        nc.tensor.matmul(
                            st[:, h, 0:nv],
                            k_sb[pr][lo:hi, si * P : (si + 1) * P],
                            q_sb[pr][lo:hi, tt * 512 + off : (tt + 1) * 512],
                            start=True,
                            stop=True,
                            tile_position=(lo, 0),
                        )
                    clock["pe"] += MM_NS * nv + PAIR_FIX
                    nc.scalar.activation(
                        ex[:, :, 0:nv], st[:, :, 0:nv],
                        mybir.ActivationFunctionType.Exp,
                        scale=0.125,
                    )
                    start = max(clock["sc"], clock["pe"] + EXP_LAG)
                    clock["sc"] = start + 2 * nv * EXP_COL + EXP_FIX
                    sc_done[(pr, tt, si)] = clock["sc"]
                    exp_hist.append(clock["sc"])
                    if m >= 0:  # mask both heads' leading triangles
                        nc.vector.tensor_mul(
                            ex[:, :, 0:P],
                            ex[:, :, 0:P],
                            tri_sb.unsqueeze(1).broadcast_to([P, 2, P]),
                        )
                    return ex, nv, off

                def av_pair(pr, tt, si, ex, nv, off, n_s):
                    for h in range(2):
                        slot = (2 * pr + h) * (K + 1)
                        nc.tensor.matmul(
                            avs[h][:, off:512],
                            v_sb[:, si, slot : slot + K + 1],
                            ex[:, h, 0:nv],
                            start=(si == 0),
                            stop=(si == n_s - 1),
                        )
                    clock["pe"] += 2 * MM_NS * nv

                def direct_normalize(pr, tt, avs):
                    for h in range(2):
                        cols = slice(tt * 512, (tt + 1) * 512)
                        sums = ap_.tile([1, 512], F32, tag="sums", bufs=6)
                        nc.vector.tensor_copy(sums, avs[h][K : K + 1, :])
                        recip = ap_.tile([1, 512], F32, tag="recip", bufs=6)
                        nc.vector.reciprocal_approx_fast(recip, sums)
                        bc_sb = ap_.tile([K, 512], F32, tag="bc_sb", bufs=6)
                        nc.gpsimd.partition_broadcast(bc_sb, recip)
                        if h == 0:
                            dst = concat_sb[0:K, pr, cols]
                        else:
                            dst = ap_.tile([K, 512], F16, tag="tmpb", bufs=6)
                        nc.vector.tensor_mul(dst, avs[h][0:K, :], bc_sb)
                        if h == 1:
                            nc.gpsimd.dma_start(
                                out=concat_sb[K:P, pr, cols], in_=dst
                            )

                def slice_normalize(pr, tt, avs, i16):
                    """128-col slice normalize for the phase's last pair,
                    so each out-proj group unlocks as early as possible."""
                    cols_lo = i16 * P
                    for h in range(2):
                        cols = slice(tt * 512 + cols_lo, tt * 512 + cols_lo + P)
                        psl = slice(cols_lo, cols_lo + P)
                        sums = ap_.tile([1, P], F32, tag="sums", bufs=6)
                        nc.vector.tensor_copy(sums, avs[h][K : K + 1, psl])
                        recip = ap_.tile([1, P], F32, tag="recip", bufs=6)
                        nc.vector.reciprocal_approx_fast(recip, sums)
                        bc_sb = ap_.tile([K, P], F32, tag="bc_sb", bufs=6)
                        nc.gpsimd.partition_broadcast(bc_sb, recip)
                        if h == 0:
                            dst = concat_sb[0:K, pr, cols]
                        else:
                            dst = ap_.tile([K, P], F16, tag="tmpb", bufs=6)
                        nc.vector.tensor_mul(dst, avs[h][0:K, psl], bc_sb)
                        if h == 1:
                            (nc.gpsimd if i16 % 2 == 0 else nc.sync).dma_start(
                                out=concat_sb[K:P, pr, cols], in_=dst
                            )

                # ---------------- startup -----------------------------
                warm_ps = psa.tile([P, 512], F32, tag="mm", bufs=2, name="warm")

                def warm(n):
                    for _ in range(n):
                        nc.tensor.matmul(
                            warm_ps, warm_sb[:, 0:P], warm_sb,
                            start=True, stop=True,
                        )

                warm(6)
                # p0's projections emitted directly in DMA-arrival order
                # (chunks alternate between the two fast queues); scores
                # for (p0, tt0) then start while everything else streams.
                for which in ("q", "k"):
                    ops = proj_unit_ops(0, 0, which)
                    for c in (0, 4, 1, 5, 2, 6, 3, 7):
                        fn, cost = ops[c]
                        fn()
                        clock["pe"] += cost
                    ops[DC][0]()  # cast

                # fill inventory, deadline = segment index (tt*4+pr).
                # ready_ns = rough DMA landing estimate for the op's data.
                for s in range(4):
                    fill_extend(0, v_group_ops(s), ready=23000, kind="av")
                for pr in range(1, NPAIR):
                    rdy = {1: 17000, 2: 26000, 3: 28000}[pr]
                    fill_extend(pr, proj_unit_ops(pr, 0, "q"), ready=rdy)
                    fill_extend(pr, proj_unit_ops(pr, 0, "k"), ready=rdy)
                for tt in range(1, NT):
                    xt_rdy = [0, 34000, 42000, 30000][tt]
                    for s in range(4 * tt, 4 * tt + 4):
                        fill_extend(
                            4 * tt, v_group_ops(s), ready=xt_rdy, kind="av"
                        )
                    for pr in range(NPAIR):
                        fill_extend(
                            4 * tt + pr, proj_unit_ops(pr, tt, "q"), ready=xt_rdy
                        )
                        fill_extend(
                            4 * tt + pr, proj_unit_ops(pr, tt, "k"), ready=xt_rdy
                        )

                # ---------------- main tt-major loop ------------------
                for tt in range(NT):
                    n_s = 4 * tt + 4
                    for pr in range(NPAIR):
                        seg = 4 * tt + pr
                        flush_due(seg)
                        avs = [
                            psa.tile(
                                [K + 1, 512], F32, tag="av", bufs=2,
                                name=f"av{pr}_{tt}_{h2}",
                            )
                            for h2 in range(2)
                        ]
                        # AV trails the score stream by 2 steps so an AV
                        # at the PE queue head never waits on its exp.
                        pend = {}
                        for si in range(n_s):
                            # score pair needs the st slot freed by the
                            # exp two score-pairs back (global rotation)
                            pace_to(exp_hist[-2] + 100)
                            pend[si] = score_exp(pr, tt, si)
                            if si == 1:
                                # V tiles must be in the stream before
                                # the segment's first AV
                                flush_due(seg, kinds=("pre", "av"))
                            if si >= 2:
                                pace_to(sc_done[(pr, tt, si - 2)] + 80)
                                av_pair(pr, tt, si - 2, *pend.pop(si - 2), n_s)
                        for sj in (n_s - 2, n_s - 1):
                            pace_to(sc_done[(pr, tt, sj)] + 80)
                            av_pair(pr, tt, sj, *pend.pop(sj), n_s)
                        if pr == NPAIR - 1:
                            last = tt == NT - 1
                            if last:
                                flush_mode[0] = True
                            for i16 in range(4):
                                slice_normalize(pr, tt, avs, i16)
                                t16 = 4 * tt + i16
                                fill_extend(99, op_group_ops(t16, 0))
                                fill_extend(99, op_group_ops(t16, 1))
                                if last:
                                    # keep the PE activity window dense
                                    # while the concat bounce flies so
                                    # the HAM clock stays at 8/8 through
                                    # the flush
                                    warm(2)
                                    for _ in range(5):
                                        pop_fill()
                                    warm(1)
                                    for _ in range(5):
                                        pop_fill()
                        else:
                            direct_normalize(pr, tt, avs)

                # ---------------- flush -------------------------------
                flush_mode[0] = True
                while fillq:
                    pop_fill()
    _fuse_score_ldweights(nc)
    nc.compile()
    return nc


def _fuse_score_ldweights(nc):
    """Merge each score pair's two 64-row LDWEIGHTS into one 128-row load.

    The post-Tile IR carries [Ldw(h0 64p), MM(0,0), Ldw(h1 64p), MM(64,0)]
    per key tile. With two LDWs the PE stalls ~100ns on each side of the
    pair (single background weight buffer). One 128-row LDW loads both
    heads' K slice at once; the row-tiled matmuls then address their own
    row groups of the already-loaded array.
    """
    fn = list(nc.m.functions)[0]
    fused = 0
    for blk in fn.blocks:
        insts = blk.instructions
        # pattern-match on the PE-engine subsequence: other engines'
        # instructions interleave freely in the block list
        pe = [
            (i, x)
            for i, x in enumerate(insts)
            if type(x).__name__ in ("InstLdweights", "InstMatmult")
        ]
        drop = []
        for k in range(len(pe) - 3):
            (_, a), (_, b), (ic, c), (_, d) = pe[k], pe[k + 1], pe[k + 2], pe[k + 3]
            if not (
                type(a).__name__ == "InstLdweights"
                and type(b).__name__ == "InstMatmult"
                and type(c).__name__ == "InstLdweights"
                and type(d).__name__ == "InstMatmult"
            ):
                continue
            if not (
                tuple(b.tile_size or ()) == (64, 128)
                and tuple(b.tile_position or ()) == (0, 0)
                and tuple(d.tile_size or ()) == (64, 128)
                and tuple(d.tile_position or ()) == (64, 0)
            ):
                continue
            apA, apC = a.ins[0], c.ins[0]
            pa, pc = list(apA.ap), list(apC.ap)
            if not (
                len(pa) == 2
                and pa[0][1] == 64
                and pc[0][1] == 64
                and pa[0][0] == pc[0][0]
                and pa[1] == pc[1]
                and apC.offset == apA.offset + 64 * pa[0][0]
                and c.sync_info is None
            ):
                continue
            apA.ap = [[pa[0][0], 128], pa[1]]
            if tuple(a.tile_size or ()) == (64, 128):
                a.tile_size = (128, 128)
            a.merge_dependencies_from(c)
            drop.append(ic)
            fused += 1
        for j in sorted(drop, reverse=True):
            del insts[j]
    assert fused > 0, "score LDW fusion matched nothing"


def shard_inputs(X, Wq, Wk, Wv, Wo):
    """Host-side shard prep: core c handles batch c//2, head group c%2."""
    in_maps = []
    for c in range(8):
        b, g = c // 2, c % 2
        heads = range(g * HG, (g + 1) * HG)
        wq = np.stack(
            [
                np.concatenate([Wq[g * HG + 2 * p], Wq[g * HG + 2 * p + 1]], axis=1)
                for p in range(NPAIR)
            ]
        )
        wk = np.stack(
            [
                np.concatenate([Wk[g * HG + 2 * p], Wk[g * HG + 2 * p + 1]], axis=1)
                for p in range(NPAIR)
            ]
        )
        wv = np.concatenate([Wv[h] for h in heads], axis=1)
        wo = Wo[:, g * 512 : (g + 1) * 512].T
        in_maps.append(
            {
                "xt": np.ascontiguousarray(X[b].T).astype(np.float16),
                "wq": np.ascontiguousarray(wq).astype(np.float16),
                "wk": np.ascontiguousarray(wk).astype(np.float16),
                "wv": np.ascontiguousarray(wv).astype(np.float16),
                "wo": np.ascontiguousarray(wo).astype(np.float16),
            }
        )
    return in_maps


_MODULE = None


def _get_module():
    global _MODULE
    if _MODULE is None:
        _MODULE = build_module()
    return _MODULE


def kernel(X, Wq, Wk, Wv, Wo, bo, _want_results=None):
    from concourse.bass_utils import run_bass_kernel_spmd

    nc = _get_module()
    in_maps = shard_inputs(
        np.asarray(X), np.asarray(Wq), np.asarray(Wk), np.asarray(Wv), np.asarray(Wo)
    )
    res = run_bass_kernel_spmd(nc, in_maps, core_ids=list(range(8)))
    if _want_results is not None:
        _want_results.append(res)
    out = np.empty((B, T, H * K), dtype=np.float32)
    bo = np.asarray(bo, dtype=np.float32)
    for b in range(B):
        out[b] = (
            res.results[2 * b]["out"].astype(np.float32)
            + res.results[2 * b + 1]["out"].astype(np.float32)
            + bo
        )
    return out
